# revision 1
# baseline (speedup 1.0000x reference)
"""Trainium2 Bass kernel for nn_BasicTransformerBlock_35304631173827.

Sharding: 8 cores = 4 samples x 2 sequence halves. Each core computes its
1024-token half of one sample fully locally (self-attention K/V recomputed
over the full 2048-token sample -> zero collectives). bf16 matmuls with
fp32 PSUM accumulation; LayerNorm stats and softmax in fp32.

v2: SBUF-resident residual stream (x -> x1 -> x2 stay on-chip), per-chunk
streamed attention (scores->exp->AV software pipeline, softmax normalize
via tensor_tensor divide), biases applied as K=1 rank-1 matmuls, GEGLU
fused with scalar_tensor_tensor, FF2 accumulator seeded with the residual,
phase-0 interleaved with LN1 and weight prefetch.
"""

import numpy as np
import ml_dtypes

BF16 = ml_dtypes.bfloat16

B, N, D = 4, 2048, 1024
J, CD = 256, 768
H, DH = 16, 64
INNER = 1024
FF = 4096
P = 128
KT = D // P            # 8
CKT = CD // P          # 6
TT_FULL = N // P       # 16
N_OWN = N // 2
TT_OWN = N_OWN // P    # 8
EPS = 1e-5

_CACHE = {}


def _build_program():
    import concourse.tile as tile
    from concourse import mybir, bacc
    from concourse.masks import make_identity
    from contextlib import ExitStack

    f32 = mybir.dt.float32
    bf16 = mybir.dt.bfloat16
    AF = mybir.ActivationFunctionType
    ALU = mybir.AluOpType

    nc = bacc.Bacc(None, target_bir_lowering=False)

    xf_d = nc.dram_tensor("xf", [TT_FULL, P, D], f32, kind="ExternalInput")
    tT_d = nc.dram_tensor("tT", [P, KT], bf16, kind="ExternalInput")
    nw_d = nc.dram_tensor("nw", [P, KT, 6 * D], bf16, kind="ExternalInput")
    nbc_d = nc.dram_tensor("nbc", [P, 48], f32, kind="ExternalInput")
    wq1_d = nc.dram_tensor("wq1", [P, KT, INNER], bf16, kind="ExternalInput")
    wk1_d = nc.dram_tensor("wk1", [P, KT, INNER], bf16, kind="ExternalInput")
    wv1_d = nc.dram_tensor("wv1", [P, KT, INNER], bf16, kind="ExternalInput")
    wo1_d = nc.dram_tensor("wo1", [P, KT, D], bf16, kind="ExternalInput")
    wq2_d = nc.dram_tensor("wq2", [P, KT, INNER], bf16, kind="ExternalInput")
    wk2_d = nc.dram_tensor("wk2", [P, CKT, INNER], bf16, kind="ExternalInput")
    wv2_d = nc.dram_tensor("wv2", [P, CKT, INNER], bf16, kind="ExternalInput")
    wo2_d = nc.dram_tensor("wo2", [P, KT, D], bf16, kind="ExternalInput")
    ctxT_d = nc.dram_tensor("ctxT", [P, CKT, J], bf16, kind="ExternalInput")
    brow_d = nc.dram_tensor("brow", [1, 3 * D], bf16, kind="ExternalInput")
    fb1_d = nc.dram_tensor("fb1c", [P, 64], f32, kind="ExternalInput")
    wf1_d = nc.dram_tensor("wf1", [P, KT, 2 * FF], bf16, kind="ExternalInput")
    wf2_d = nc.dram_tensor("wf2", [P, FF // P, D], bf16, kind="ExternalInput")
    y_d = nc.dram_tensor("y", [TT_OWN, P, D], f32, kind="ExternalOutput")

    # DRAM scratch for the full-sample transposed LN1 output and K^T.
    h1T_dram = nc.dram_tensor("s_h1T", [P, KT, N_OWN], bf16, kind="Internal")
    kT_dram = nc.dram_tensor("s_kT", [KT, P, N], bf16, kind="Internal")

    with tile.TileContext(nc) as tc, ExitStack() as es:
        konst = es.enter_context(tc.tile_pool(name="konst", bufs=1))
        xpool = es.enter_context(tc.tile_pool(name="xpool", bufs=1))
        xres = es.enter_context(tc.tile_pool(name="xres", bufs=8))
        stats = es.enter_context(tc.tile_pool(name="stats", bufs=2))
        wres = es.enter_context(tc.tile_pool(name="wres", bufs=2))
        wsm = es.enter_context(tc.tile_pool(name="wsm", bufs=3))
        wmed = es.enter_context(tc.tile_pool(name="wmed", bufs=1))
        evict = es.enter_context(tc.tile_pool(name="evict", bufs=2))
        stg = es.enter_context(tc.tile_pool(name="stg", bufs=2))
        big = es.enter_context(tc.tile_pool(name="big", bufs=2))
        vpool = es.enter_context(tc.tile_pool(name="vpool", bufs=1))
        kthp = es.enter_context(tc.tile_pool(name="kthp", bufs=1))
        expp = es.enter_context(tc.tile_pool(name="expp", bufs=3))
        ps_a = es.enter_context(tc.tile_pool(name="ps_a", bufs=2, space="PSUM"))
        ps_sc = es.enter_context(tc.tile_pool(name="ps_sc", bufs=2, space="PSUM"))
        ps_av = es.enter_context(tc.tile_pool(name="ps_av", bufs=2, space="PSUM"))
        ps_tr = ps_av

        # ---------------- constants ----------------
        ident = konst.tile([P, P], bf16)
        make_identity(nc, ident)
        ones64 = konst.tile([1, 64], bf16)
        nc.vector.memset(ones64[:], 1.0)
        ones1 = konst.tile([1, P], bf16)
        nc.vector.memset(ones1[:], 1.0)
        eps_t = konst.tile([P, 1], f32)
        nc.vector.memset(eps_t[:], EPS)
        tT_sb = konst.tile([P, KT], bf16)
        nc.sync.dma_start(tT_sb[:], tT_d[:])
        nbc_sb = konst.tile([P, 48], f32)
        nc.sync.dma_start(nbc_sb[:], nbc_d[:])
        fb1_sb = konst.tile([P, 64], f32)
        nc.sync.dma_start(fb1_sb[:], fb1_d[:])
        brow_sb = konst.tile([1, 3 * D], bf16)
        nc.sync.dma_start(brow_sb[:], brow_d[:])
        ctxT_sb = konst.tile([P, CKT, J], bf16)
        nc.sync.dma_start(ctxT_sb[:], ctxT_d[:])
        cols = konst.tile([P, 48], f32)

        # ---------------- phase 0 pieces (AdaLN embedding) ----------------
        def p0_chunk(c):
            nwt = wsm.tile([P, KT, P], bf16, tag="wstream")
            nc.sync.dma_start(nwt[:], nw_d[:, :, c * P:(c + 1) * P])
            ps = ps_a.tile([P, 512], f32, tag="psa")
            for kt in range(KT):
                nc.tensor.matmul(ps[:, 0:1], nwt[:, kt, :], tT_sb[:, kt:kt + 1],
                                 start=(kt == 0), stop=(kt == KT - 1))
            nc.vector.tensor_copy(cols[:, c:c + 1], ps[:, 0:1])

        def p0_fixup(n3):
            # cols[n3 group] += bias; scale part += 1.0
            sl = slice(n3 * 16, (n3 + 1) * 16)
            nc.vector.tensor_add(cols[:, sl], cols[:, sl], nbc_sb[:, sl])
            nc.vector.tensor_scalar_add(cols[:, n3 * 16:n3 * 16 + 8],
                                        cols[:, n3 * 16:n3 * 16 + 8], 1.0)

        def layernorm_tile(x_tile, tt, n3, dst_sb=None, stage=None, soff=0):
            """LayerNorm + AdaLN affine on (P, D) tile -> transposed chunks."""
            bst = stats.tile([P, 2, 6], f32, tag="bnst")
            for g in range(2):
                nc.vector.bn_stats(bst[:, g, :], x_tile[:, g * 512:(g + 1) * 512])
            mv = stats.tile([P, 4], f32, tag="mv")
            nc.vector.bn_aggr(mv[:, 0:2], bst[:])
            nc.scalar.activation(mv[:, 2:3], mv[:, 1:2], AF.Sqrt, bias=eps_t[:])
            nc.vector.reciprocal(mv[:, 2:3], mv[:, 2:3])
            nc.vector.tensor_tensor(mv[:, 3:4], mv[:, 0:1], mv[:, 2:3], ALU.mult)
            nc.vector.tensor_scalar_mul(mv[:, 3:4], mv[:, 3:4], -1.0)
            xn = evict.tile([P, D], bf16, tag="xn")
            nc.scalar.activation(xn[:], x_tile[:], AF.Identity,
                                 bias=mv[:, 3:4], scale=mv[:, 2:3])
            for c in range(KT):
                pt = ps_tr.tile([P, P], bf16, tag="psav")
                nc.tensor.transpose(pt[:], xn[:, c * P:(c + 1) * P], ident[:])
                out_ap = (stage[:, c, soff:soff + P] if stage is not None
                          else dst_sb[:, c, tt * P:(tt + 1) * P])
                nc.vector.tensor_scalar(
                    out_ap, pt[:],
                    cols[:, n3 * 16 + c:n3 * 16 + c + 1],
                    cols[:, n3 * 16 + 8 + c:n3 * 16 + 8 + c + 1],
                    ALU.mult, ALU.add)

        # ------- Phase 0+1+2: LN1 fused with K/V projections -------------
        SC = DH ** -0.5
        for c in range(16):
            p0_chunk(c)
        p0_fixup(0)

        wk1_sb = wres.tile([P, KT, INNER], bf16, tag="wbig")
        nc.sync.dma_start(wk1_sb[:], wk1_d[:])
        wv1_sb = wres.tile([P, KT, INNER], bf16, tag="wbig")
        nc.sync.dma_start(wv1_sb[:], wv1_d[:])

        v_sb = vpool.tile([P, TT_FULL, H, DH + 1], bf16, tag="v33")
        nc.vector.memset(v_sb[:, :, :, DH:DH + 1], 1.0)

        x_own = []
        stage2 = None
        for tt in range(TT_FULL):
            if tt < TT_OWN:
                xt = xres.tile([P, D], f32, tag="xr")
                x_own.append(xt)
            else:
                xt = xpool.tile([P, D], f32, tag="x")
            nc.sync.dma_start(xt[:], xf_d[tt])
            if tt % 2 == 0:
                stage2 = stg.tile([P, KT, 256], bf16, tag="stage", name="stage")
            layernorm_tile(xt, tt, 0, stage=stage2, soff=(tt % 2) * P)
            # V projection for this tile straight from the staged LN output
            hch = stage2[:, :, (tt % 2) * P:(tt % 2) * P + P]
            for nc2 in range(2):
                ps = ps_a.tile([P, 512], f32, tag="psa")
                for kt in range(KT):
                    nc.tensor.matmul(ps[:], hch[:, kt, :],
                                     wv1_sb[:, kt, nc2 * 512:(nc2 + 1) * 512],
                                     start=(kt == 0), stop=(kt == KT - 1))
                nc.vector.tensor_copy(
                    v_sb[:, tt, nc2 * 8:(nc2 + 1) * 8, 0:DH],
                    ps[:].rearrange("p (hh r) -> p hh r", r=DH))
            if tt % 2 == 1:
                c2 = tt // 2
                # K chunk for 256 tokens from the staged pair
                for m in range(KT):
                    ps = ps_a.tile([P, 512], f32, tag="psa")
                    for kt in range(KT):
                        nc.tensor.matmul(ps[:, 0:256],
                                         wk1_sb[:, kt, m * P:(m + 1) * P],
                                         stage2[:, kt, :],
                                         start=(kt == 0), stop=(kt == KT - 1))
                    kst = stg.tile([P, 256], bf16, tag="kstage")
                    nc.vector.tensor_copy(kst[:], ps[:, 0:256])
                    nc.sync.dma_start(
                        kT_dram[m, :, c2 * 256:(c2 + 1) * 256], kst[:])
                if tt < TT_OWN:
                    # own-half transposed LN1 out to DRAM for the Q pass
                    nc.sync.dma_start(
                        h1T_dram[:, :, c2 * 256:(c2 + 1) * 256], stage2[:])
            # stream remaining phase-0 chunks behind LN1 tiles
            for c in range(16 + tt * 2, min(16 + tt * 2 + 2, 48)):
                p0_chunk(c)
            if tt == 7:
                p0_fixup(1)
            if tt == 15:
                p0_fixup(2)

        # ---------------- Q projection (own half) ----------------
        qT = big.tile([P, KT, N_OWN], bf16, tag="t2m")

        def qk_proj(w_dram, n_tok, out_sb, out_dram, scale):
            w_sb = wres.tile([P, KT, INNER], bf16, tag="wbig")
            nc.sync.dma_start(w_sb[:], w_dram[:])
            for qc in range(n_tok // 256):
                hch = wmed.tile([P, KT, 256], bf16, tag="med4")
                nc.sync.dma_start(hch[:], h1T_dram[:, :, qc * 256:(qc + 1) * 256])
                for m in range(KT):
                    ps = ps_a.tile([P, 512], f32, tag="psa")
                    for kt in range(KT):
                        nc.tensor.matmul(ps[:, 0:256],
                                         w_sb[:, kt, m * P:(m + 1) * P],
                                         hch[:, kt, :],
                                         start=(kt == 0), stop=(kt == KT - 1))
                    if out_sb is not None:
                        nc.vector.tensor_scalar_mul(
                            out_sb[:, m, qc * 256:(qc + 1) * 256], ps[:, 0:256], scale)
                    else:
                        kst = stg.tile([P, 256], bf16, tag="kstage")
                        nc.vector.tensor_copy(kst[:], ps[:, 0:256])
                        nc.sync.dma_start(
                            out_dram[m, :, qc * 256:(qc + 1) * 256], kst[:])

        qk_proj(wq1_d, N_OWN, qT, None, SC)

        # ---------------- attention (shared for self / cross) ----------------
        def attention(get_k, v_t, qT_t, n_keys_tt, out_T):
            nkk = min(2, n_keys_tt)
            den = None
            grp_meta = []
            for h in range(H):
                hp = (h % 2) * 64
                m2 = h // 2
                kap = get_k(h)  # (P, n_keys) tile; head at partitions hp:hp+64
                for qc in range(2):
                    gi = (h * 2 + qc) % 4
                    if gi == 0:
                        den = stats.tile([P, 512], bf16, tag="den")
                        nc.gpsimd.memset(den[:], 1.0)
                        grp_meta = []
                    pavt = ps_av.tile([P, 512], f32, tag="psav")
                    for kt2 in range(max(1, n_keys_tt // 2)):
                        ps_s = ps_sc.tile([P, 1024], f32, tag="pssc")
                        for u in range(nkk):
                            kt = kt2 * 2 + u
                            nc.tensor.matmul(
                                ps_s[:, u * 512:(u + 1) * 512],
                                kap[hp:hp + 64, kt * P:(kt + 1) * P],
                                qT_t[hp:hp + 64, m2, qc * 512:(qc + 1) * 512],
                                start=True, stop=True)
                        ex = expp.tile([P, 2, 512], bf16, tag="expT")
                        nc.scalar.activation(
                            ex[:, 0:nkk, :].rearrange("p a b -> p (a b)"),
                            ps_s[:, 0:nkk * 512], AF.Exp)
                        for u in range(nkk):
                            kt = kt2 * 2 + u
                            fl = dict(start=(kt == 0), stop=(kt == n_keys_tt - 1),
                                      skip_group_check=True)
                            nc.tensor.matmul(pavt[0:DH + 1], v_t[:, kt, h, :],
                                             ex[:, u, :], **fl)
                    # unnormalized AV and denominator out of PSUM
                    nc.vector.tensor_copy(
                        out_T[hp:hp + 64, m2, qc * 512:(qc + 1) * 512],
                        pavt[0:DH, :])
                    nc.vector.tensor_copy(den[32 * gi:32 * gi + 1, :],
                                          pavt[DH:DH + 1, :])
                    grp_meta.append((h, qc, 32 * gi))
                    if gi == 3:
                        denr = stats.tile([P, 512], bf16, tag="denr")
                        with nc.allow_low_precision(reason="softmax denom"):
                            nc.vector.reciprocal(denr[:], den[:])
                        for (hh, qq, base) in grp_meta:
                            hp2 = (hh % 2) * 64
                            mm2 = hh // 2
                            sl2 = slice(qq * 512, (qq + 1) * 512)
                            dr0 = stats.tile([1, 512], bf16, tag="dr0")
                            nc.scalar.copy(dr0[:], denr[base:base + 1, :])
                            bcs = stats.tile([P, 512], bf16, tag="bcs")
                            nc.gpsimd.partition_broadcast(bcs[:], dr0[:],
                                                          channels=P)
                            nc.vector.tensor_tensor(
                                out_T[hp2:hp2 + 64, mm2, sl2],
                                bcs[hp2:hp2 + 64, :],
                                out_T[hp2:hp2 + 64, mm2, sl2], ALU.mult)

        # ---------------- Phase 3: self-attention ----------------
        attn1T = big.tile([P, KT, N_OWN], bf16, tag="t2m")
        _kcache = {}

        def get_k_self(h):
            m2 = h // 2
            if m2 not in _kcache:
                kth = kthp.tile([P, N], bf16, tag="kTh", name="kth")
                nc.sync.dma_start(kth[:], kT_dram[m2])
                _kcache.clear()
                _kcache[m2] = kth
            return _kcache[m2]

        attention(get_k_self, v_sb, qT, TT_FULL, attn1T)

        # ---------------- o-proj + residual + LN (fused per tile) ----------
        def out_proj_ln(attn_T, w_sb, bias_idx, res_tiles, n3, dst_sb):
            # x_res <- o_proj(attn) + bias + x_res (in place); LN(n3) -> dst_sb
            for tt in range(TT_OWN):
                xt = res_tiles[tt]
                for dch in range(2):
                    ps = ps_a.tile([P, 512], f32, tag="psa")
                    for m in range(KT):
                        nc.tensor.matmul(ps[:],
                                         attn_T[:, m, tt * P:(tt + 1) * P],
                                         w_sb[:, m, dch * 512:(dch + 1) * 512],
                                         start=(m == 0), stop=False)
                    nc.tensor.matmul(
                        ps[:], ones1[:],
                        brow_sb[:, bias_idx * D + dch * 512:
                                bias_idx * D + (dch + 1) * 512],
                        start=False, stop=True)
                    nc.vector.tensor_tensor(
                        xt[:, dch * 512:(dch + 1) * 512], ps[:],
                        xt[:, dch * 512:(dch + 1) * 512], ALU.add)
                layernorm_tile(xt, tt, n3, dst_sb=dst_sb)

        wo1_sb = wres.tile([P, KT, INNER], bf16, tag="wbig")
        nc.sync.dma_start(wo1_sb[:], wo1_d[:])
        h2T = big.tile([P, KT, N_OWN], bf16, tag="t2m")
        out_proj_ln(attn1T, wo1_sb, 0, x_own, 1, h2T)

        # ---------------- Phase 5: q2 ----------------
        q2T = big.tile([P, KT, N_OWN], bf16, tag="t2m")
        w_sb = wres.tile([P, KT, INNER], bf16, tag="wbig")
        nc.sync.dma_start(w_sb[:], wq2_d[:])
        for m in range(KT):
            for qc in range(2):
                ps = ps_a.tile([P, 512], f32, tag="psa")
                for kt in range(KT):
                    nc.tensor.matmul(ps[:], w_sb[:, kt, m * P:(m + 1) * P],
                                     h2T[:, kt, qc * 512:(qc + 1) * 512],
                                     start=(kt == 0), stop=(kt == KT - 1))
                nc.vector.tensor_scalar_mul(q2T[:, m, qc * 512:(qc + 1) * 512],
                                            ps[:], SC)

        # ---------------- Phase 6: cross-attention ----------------
        k2T = kthp.tile([P, KT, J], bf16, tag="kTh", name="k2T")
        w_sb = wres.tile([P, CKT, INNER], bf16, tag="wbig")
        nc.sync.dma_start(w_sb[:], wk2_d[:])
        for m in range(KT):
            ps = ps_a.tile([P, 512], f32, tag="psa")
            for kt in range(CKT):
                nc.tensor.matmul(ps[:, 0:J], w_sb[:, kt, m * P:(m + 1) * P],
                                 ctxT_sb[:, kt, :],
                                 start=(kt == 0), stop=(kt == CKT - 1))
            nc.vector.tensor_copy(k2T[:, m, :], ps[:, 0:J])
        v2_sb = vpool.tile([P, J // P, H, DH + 1], bf16, tag="v33", name="v2_sb")
        nc.vector.memset(v2_sb[:, :, :, DH:DH + 1], 1.0)
        w_sb = wres.tile([P, CKT, INNER], bf16, tag="wbig")
        nc.sync.dma_start(w_sb[:], wv2_d[:])
        for tt in range(J // P):
            for nc2 in range(2):
                ps = ps_a.tile([P, 512], f32, tag="psa")
                for kt in range(CKT):
                    nc.tensor.matmul(ps[:], ctxT_sb[:, kt, tt * P:(tt + 1) * P],
                                     w_sb[:, kt, nc2 * 512:(nc2 + 1) * 512],
                                     start=(kt == 0), stop=(kt == CKT - 1))
                nc.vector.tensor_copy(
                    v2_sb[:, tt, nc2 * 8:(nc2 + 1) * 8, 0:DH],
                    ps[:].rearrange("p (hh r) -> p hh r", r=DH))

        attn2T = big.tile([P, KT, N_OWN], bf16, tag="t2m")

        def get_k_cross(h):
            return k2T[:, h // 2, :]

        attention(get_k_cross, v2_sb, q2T, J // P, attn2T)

        wo2_sb = wres.tile([P, KT, INNER], bf16, tag="wbig")
        nc.sync.dma_start(wo2_sb[:], wo2_d[:])
        h3T = big.tile([P, KT, N_OWN], bf16, tag="t2m")
        out_proj_ln(attn2T, wo2_sb, 1, x_own, 2, h3T)

        # ---------------- Phase 9: GEGLU FF ----------------
        # FF2 partials accumulate into the (dead-after-seed) x2 residual tiles.
        g_sb = big.tile([P, 8, N_OWN], bf16, tag="g_sb", bufs=1)
        for grp in range(4):
            wf2g = wres.tile([P, 8, D], bf16, tag="wbig")
            nc.sync.dma_start(wf2g[:], wf2_d[:, grp * 8:(grp + 1) * 8, :])
            for j in range(8):
                f = grp * 8 + j
                wa = wsm.tile([P, KT, P], bf16, tag="wstream")
                nc.sync.dma_start(wa[:], wf1_d[:, :, f * P:(f + 1) * P])
                wg = wsm.tile([P, KT, P], bf16, tag="wstream")
                nc.sync.dma_start(wg[:], wf1_d[:, :, FF + f * P:FF + (f + 1) * P])
                gt_sb = evict.tile([P, N_OWN], bf16, tag="gt_sb")
                for qc in range(2):
                    sl = slice(qc * 512, (qc + 1) * 512)
                    ps2 = ps_sc.tile([P, 1024], f32, tag="pssc")
                    for kt in range(KT):
                        nc.tensor.matmul(ps2[:, 0:512], wg[:, kt, :], h3T[:, kt, sl],
                                         start=(kt == 0), stop=(kt == KT - 1))
                    nc.scalar.activation(gt_sb[:, sl], ps2[:, 0:512], AF.Gelu,
                                         bias=fb1_sb[:, 32 + f:32 + f + 1])
                    ps1 = ps_sc.tile([P, 1024], f32, tag="pssc")
                    for kt in range(KT):
                        nc.tensor.matmul(ps1[:, 0:512], wa[:, kt, :], h3T[:, kt, sl],
                                         start=(kt == 0), stop=(kt == KT - 1))
                    # g = (a + b1a) * gelu(gate)
                    nc.vector.scalar_tensor_tensor(
                        g_sb[:, j, sl], ps1[:, 0:512], fb1_sb[:, f:f + 1],
                        gt_sb[:, sl], ALU.add, ALU.mult)
            for tt in range(TT_OWN):
                for dc in range(2):
                    sl = slice(dc * 512, (dc + 1) * 512)
                    ps = ps_a.tile([P, 512], f32, tag="psa")
                    for jj in range(8):
                        nc.tensor.matmul(ps[:], g_sb[:, jj, tt * P:(tt + 1) * P],
                                         wf2g[:, jj, sl],
                                         start=(jj == 0),
                                         stop=(jj == 7 and grp != 3))
                    if grp < 3:
                        nc.vector.tensor_tensor(
                            x_own[tt][:, sl], x_own[tt][:, sl], ps[:], ALU.add)
                    else:
                        # last group: fold in ff bias via rank-1 matmul, then
                        # final accumulate in fp32 and store out.
                        nc.tensor.matmul(
                            ps[:], ones1[:],
                            brow_sb[:, 2 * D + dc * 512:2 * D + (dc + 1) * 512],
                            start=False, stop=True)
                        yt = evict.tile([P, 512], f32, tag="yt", bufs=1)
                        nc.vector.tensor_tensor(yt[:], x_own[tt][:, sl], ps[:],
                                                ALU.add)
                        nc.sync.dma_start(y_d[tt, :, sl], yt[:])

    nc.compile()
    return nc


def _rearr_w(w, kt):
    return np.ascontiguousarray(
        w.reshape(kt, P, -1).transpose(1, 0, 2)).astype(BF16)


def _shard_inputs(inputs):
    f = {k: np.asarray(v, dtype=np.float32) for k, v in inputs.items()}
    shared = {
        "nw": _rearr_w(np.concatenate([f["n1_w"], f["n2_w"], f["n3_w"]], axis=1), KT),
        "nbc": np.ascontiguousarray(
            np.concatenate([f["n1_b"], f["n2_b"], f["n3_b"]])
            .reshape(3, 16, P).transpose(2, 0, 1).reshape(P, 48)),
        "wq1": _rearr_w(f["q1"], KT), "wk1": _rearr_w(f["k1"], KT),
        "wv1": _rearr_w(f["v1"], KT), "wo1": _rearr_w(f["o1_w"], KT),
        "wq2": _rearr_w(f["q2"], KT), "wk2": _rearr_w(f["k2"], CKT),
        "wv2": _rearr_w(f["v2"], CKT), "wo2": _rearr_w(f["o2_w"], KT),
        "brow": np.ascontiguousarray(
            np.concatenate([f["o1_b"], f["o2_b"], f["ff_b2"]])[None]).astype(BF16),
        "fb1c": np.ascontiguousarray(f["ff_b1"].reshape(64, P).T),
        "wf1": _rearr_w(f["ff_w1"], KT),
        "wf2": _rearr_w(f["ff_w2"], FF // P),
    }
    in_maps = []
    for core in range(8):
        b, half = core // 2, core % 2
        own = f["x"][b, half * N_OWN:(half + 1) * N_OWN]
        oth = f["x"][b, (1 - half) * N_OWN:(2 - half) * N_OWN]
        m = dict(shared)
        m["xf"] = np.ascontiguousarray(
            np.concatenate([own, oth]).reshape(TT_FULL, P, D))
        m["tT"] = np.ascontiguousarray(f["t"][b, 0].reshape(KT, P).T).astype(BF16)
        m["ctxT"] = np.ascontiguousarray(
            f["context"][b].T.reshape(CKT, P, J).transpose(1, 0, 2)).astype(BF16)
        in_maps.append(m)
    return in_maps


def kernel(**inputs):
    from concourse.bass_utils import run_bass_kernel_spmd
    if "nc" not in _CACHE:
        _CACHE["nc"] = _build_program()
    nc = _CACHE["nc"]
    in_maps = _shard_inputs(inputs)
    res = run_bass_kernel_spmd(nc, in_maps, core_ids=list(range(8)))
    out = np.empty((B, N, D), dtype=np.float32)
    for core in range(8):
        b, half = core // 2, core % 2
        out[b, half * N_OWN:(half + 1) * N_OWN] = \
            res.results[core]["y"].reshape(N_OWN, D)
    return out



# revision 9
# speedup vs baseline: 1.0417x; 1.0417x over previous
"""Trainium2 Bass kernel for nn_BasicTransformerBlock_35304631173827.

Sharding: 8 cores = 4 samples x 2 sequence halves. Each core computes its
1024-token half of one sample fully locally (self-attention K/V recomputed
over the full 2048-token sample -> zero collectives).

v3: fp8(e4m3) DoubleRow matmuls for Q/K/V/O2 projections (weights pre-scaled
x256 on host, descaled 1/256 at PSUM eviction), attention score matmuls
interleaved across head pairs (PE row-halves 0/64 pipeline concurrently),
softmax scale folded into the exp activation, and a PE-broadcast based
softmax normalization (den rows -> reciprocal -> sel2 matmul broadcast ->
two 64-partition multiplies on vector/gpsimd).
"""

import numpy as np
import ml_dtypes

BF16 = ml_dtypes.bfloat16
F8E4 = ml_dtypes.float8_e4m3

B, N, D = 4, 2048, 1024
J, CD = 256, 768
H, DH = 16, 64
INNER = 1024
FF = 4096
P = 128
KT = D // P            # 8
CKT = CD // P          # 6
TT_FULL = N // P       # 16
N_OWN = N // 2
TT_OWN = N_OWN // P    # 8
EPS = 1e-5
WS = 256.0             # fp8 weight pre-scale (exact power of 2)
ISC = 1.0 / WS

_CACHE = {}


def _build_program():
    import concourse.tile as tile
    from concourse import mybir, bacc
    from concourse.masks import make_identity
    from contextlib import ExitStack

    f32 = mybir.dt.float32
    bf16 = mybir.dt.bfloat16
    f8 = mybir.dt.float8e4
    AF = mybir.ActivationFunctionType
    ALU = mybir.AluOpType
    DRm = mybir.MatmulPerfMode.DoubleRow

    nc = bacc.Bacc(None, target_bir_lowering=False)

    xf_d = nc.dram_tensor("xf", [TT_FULL, P, D], f32, kind="ExternalInput")
    tT_d = nc.dram_tensor("tT", [P, KT], bf16, kind="ExternalInput")
    nw_d = nc.dram_tensor("nw", [P, KT, 6 * D], bf16, kind="ExternalInput")
    nbc_d = nc.dram_tensor("nbc", [P, 48], f32, kind="ExternalInput")
    wq1_d = nc.dram_tensor("wq1", [P, KT, INNER], f8, kind="ExternalInput")
    wk1_d = nc.dram_tensor("wk1", [P, KT, INNER], f8, kind="ExternalInput")
    wv1_d = nc.dram_tensor("wv1", [P, KT, INNER], f8, kind="ExternalInput")
    wo1_d = nc.dram_tensor("wo1", [P, KT, D], bf16, kind="ExternalInput")
    wq2_d = nc.dram_tensor("wq2", [P, KT, INNER], f8, kind="ExternalInput")
    wk2_d = nc.dram_tensor("wk2", [P, CKT, INNER], f8, kind="ExternalInput")
    wv2_d = nc.dram_tensor("wv2", [P, CKT, INNER], f8, kind="ExternalInput")
    wo2_d = nc.dram_tensor("wo2", [P, KT, D], f8, kind="ExternalInput")
    ctxT_d = nc.dram_tensor("ctxT", [P, CKT, J], f8, kind="ExternalInput")
    brow_d = nc.dram_tensor("brow", [1, 3 * D], bf16, kind="ExternalInput")
    fb1_d = nc.dram_tensor("fb1c", [P, 64], f32, kind="ExternalInput")
    wf1_d = nc.dram_tensor("wf1", [P, KT, 2 * FF], bf16, kind="ExternalInput")
    wf2_d = nc.dram_tensor("wf2", [P, FF // P, D], bf16, kind="ExternalInput")
    y_d = nc.dram_tensor("y", [TT_OWN, P, D], f32, kind="ExternalOutput")

    # DRAM scratch: full-sample transposed LN1 output (fp8) and K^T (fp8).
    h1T_dram = nc.dram_tensor("s_h1T", [P, KT, N_OWN], f8, kind="Internal")
    kT_dram = nc.dram_tensor("s_kT", [KT, P, N], f8, kind="Internal")

    SC = DH ** -0.5

    with tile.TileContext(nc) as tc, ExitStack() as es:
        konst = es.enter_context(tc.tile_pool(name="konst", bufs=1))
        xpool = es.enter_context(tc.tile_pool(name="xpool", bufs=1))
        xres = es.enter_context(tc.tile_pool(name="xres", bufs=8))
        stats = es.enter_context(tc.tile_pool(name="stats", bufs=2))
        wres = es.enter_context(tc.tile_pool(name="wres", bufs=2))
        wsm = es.enter_context(tc.tile_pool(name="wsm", bufs=3))
        wmed = es.enter_context(tc.tile_pool(name="wmed", bufs=1))
        evict = es.enter_context(tc.tile_pool(name="evict", bufs=2))
        stg = es.enter_context(tc.tile_pool(name="stg", bufs=2))
        big = es.enter_context(tc.tile_pool(name="big", bufs=2))
        vpool = es.enter_context(tc.tile_pool(name="vpool", bufs=1))
        kthp = es.enter_context(tc.tile_pool(name="kthp", bufs=1))
        expp = es.enter_context(tc.tile_pool(name="expp", bufs=3))
        ps_a = es.enter_context(tc.tile_pool(name="ps_a", bufs=2, space="PSUM"))
        ps_sc = es.enter_context(tc.tile_pool(name="ps_sc", bufs=2, space="PSUM"))
        ps_av = es.enter_context(tc.tile_pool(name="ps_av", bufs=2, space="PSUM"))
        ps_tr = ps_av

        # ---------------- constants ----------------
        ident = konst.tile([P, P], bf16)
        make_identity(nc, ident)
        ones1 = konst.tile([1, P], bf16)
        nc.vector.memset(ones1[:], 1.0)
        eps_t = konst.tile([P, 1], f32)
        nc.vector.memset(eps_t[:], EPS)
        sel65 = konst.tile([DH + 1, P], bf16)
        nc.vector.memset(sel65[:], 0.0)
        nc.vector.memset(sel65[0:1, 0:DH], 1.0)
        nc.vector.memset(sel65[DH:DH + 1, DH:P], 1.0)
        rec_t = konst.tile([P, 512], bf16)
        nc.vector.memset(rec_t[:], 0.0)
        tT_sb = konst.tile([P, KT], bf16)
        nc.sync.dma_start(tT_sb[:], tT_d[:])
        nbc_sb = konst.tile([P, 48], f32)
        nc.sync.dma_start(nbc_sb[:], nbc_d[:])
        fb1_sb = konst.tile([P, 64], f32)
        nc.sync.dma_start(fb1_sb[:], fb1_d[:])
        brow_sb = konst.tile([1, 3 * D], bf16)
        nc.sync.dma_start(brow_sb[:], brow_d[:])
        ctxT_sb = konst.tile([P, CKT, J], f8)
        nc.sync.dma_start(ctxT_sb[:], ctxT_d[:])
        cols = konst.tile([P, 48], f32)

        # ---------------- phase 0 pieces (AdaLN embedding) ----------------
        def p0_chunk(c):
            nwt = wsm.tile([P, KT, P], bf16, tag="wstream")
            nc.sync.dma_start(nwt[:], nw_d[:, :, c * P:(c + 1) * P])
            ps = ps_a.tile([P, 512], f32, tag="psa")
            for kt in range(KT):
                nc.tensor.matmul(ps[:, 0:1], nwt[:, kt, :], tT_sb[:, kt:kt + 1],
                                 start=(kt == 0), stop=(kt == KT - 1))
            nc.vector.tensor_copy(cols[:, c:c + 1], ps[:, 0:1])

        def p0_fixup(n3):
            sl = slice(n3 * 16, (n3 + 1) * 16)
            nc.vector.tensor_add(cols[:, sl], cols[:, sl], nbc_sb[:, sl])
            nc.vector.tensor_scalar_add(cols[:, n3 * 16:n3 * 16 + 8],
                                        cols[:, n3 * 16:n3 * 16 + 8], 1.0)

        def layernorm_tile(x_tile, tt, n3, dst_sb=None, stage=None, soff=0):
            """LayerNorm + AdaLN affine on (P, D) tile -> transposed chunks."""
            bst = stats.tile([P, 2, 6], f32, tag="bnst")
            for g in range(2):
                nc.vector.bn_stats(bst[:, g, :], x_tile[:, g * 512:(g + 1) * 512])
            mv = stats.tile([P, 4], f32, tag="mv")
            nc.vector.bn_aggr(mv[:, 0:2], bst[:])
            nc.scalar.activation(mv[:, 2:3], mv[:, 1:2], AF.Sqrt, bias=eps_t[:])
            nc.vector.reciprocal(mv[:, 2:3], mv[:, 2:3])
            nc.vector.tensor_tensor(mv[:, 3:4], mv[:, 0:1], mv[:, 2:3], ALU.mult)
            nc.vector.tensor_scalar_mul(mv[:, 3:4], mv[:, 3:4], -1.0)
            xn = evict.tile([P, D], bf16, tag="xn")
            nc.scalar.activation(xn[:], x_tile[:], AF.Identity,
                                 bias=mv[:, 3:4], scale=mv[:, 2:3])
            for c in range(KT):
                pt = ps_tr.tile([P, P], bf16, tag="psav")
                nc.tensor.transpose(pt[:], xn[:, c * P:(c + 1) * P], ident[:])
                out_ap = (stage[:, c, soff:soff + P] if stage is not None
                          else dst_sb[:, c, tt * P:(tt + 1) * P])
                nc.vector.tensor_scalar(
                    out_ap, pt[:],
                    cols[:, n3 * 16 + c:n3 * 16 + c + 1],
                    cols[:, n3 * 16 + 8 + c:n3 * 16 + 8 + c + 1],
                    ALU.mult, ALU.add)

        # ------- Phase 0+1+2: LN1 fused with K/V projections -------------
        for c in range(16):
            p0_chunk(c)
        p0_fixup(0)

        wk1_sb = wres.tile([P, KT, INNER], f8, tag="wbig")
        nc.sync.dma_start(wk1_sb[:], wk1_d[:])
        wv1_sb = wres.tile([P, KT, INNER], f8, tag="wbig")
        nc.sync.dma_start(wv1_sb[:], wv1_d[:])

        v_sb = vpool.tile([P, TT_FULL, H, DH + 1], bf16, tag="v33")
        nc.vector.memset(v_sb[:, :, :, DH:DH + 1], 1.0)

        x_own = []
        stage2 = None
        for tt in range(TT_FULL):
            if tt < TT_OWN:
                xt = xres.tile([P, D], f32, tag="xr")
                x_own.append(xt)
            else:
                xt = xpool.tile([P, D], f32, tag="x")
            nc.sync.dma_start(xt[:], xf_d[tt])
            if tt % 2 == 0:
                stage2 = stg.tile([P, KT, 256], f8, tag="stage", name="stage")
            layernorm_tile(xt, tt, 0, stage=stage2, soff=(tt % 2) * P)
            # V projection for this tile straight from the staged LN output
            off = (tt % 2) * P
            for nc2 in range(2):
                ps = ps_a.tile([P, 512], f32, tag="psa")
                for k2 in range(KT // 2):
                    nc.tensor.matmul(ps[:], stage2[:, 2 * k2:2 * k2 + 2, off:off + P],
                                     wv1_sb[:, 2 * k2:2 * k2 + 2,
                                            nc2 * 512:(nc2 + 1) * 512],
                                     start=(k2 == 0), stop=(k2 == KT // 2 - 1),
                                     perf_mode=DRm)
                nc.scalar.mul(
                    v_sb[:, tt, nc2 * 8:(nc2 + 1) * 8, 0:DH],
                    ps[:].rearrange("p (hh r) -> p hh r", r=DH), ISC)
            if tt % 2 == 1:
                c2 = tt // 2
                # K chunk for 256 tokens from the staged pair
                for m in range(KT):
                    ps = ps_a.tile([P, 512], f32, tag="psa")
                    for k2 in range(KT // 2):
                        nc.tensor.matmul(ps[:, 0:256],
                                         wk1_sb[:, 2 * k2:2 * k2 + 2, m * P:(m + 1) * P],
                                         stage2[:, 2 * k2:2 * k2 + 2, :],
                                         start=(k2 == 0), stop=(k2 == KT // 2 - 1),
                                         perf_mode=DRm)
                    kst = stg.tile([P, 256], f8, tag="kstage")
                    nc.vector.tensor_scalar_mul(kst[:], ps[:, 0:256], ISC)
                    nc.sync.dma_start(
                        kT_dram[m, :, c2 * 256:(c2 + 1) * 256], kst[:])
                if tt < TT_OWN:
                    nc.sync.dma_start(
                        h1T_dram[:, :, c2 * 256:(c2 + 1) * 256], stage2[:])
            # stream remaining phase-0 chunks behind LN1 tiles
            for c in range(16 + tt * 2, min(16 + tt * 2 + 2, 48)):
                p0_chunk(c)
            if tt == 7:
                p0_fixup(1)
            if tt == 15:
                p0_fixup(2)

        # ---------------- Q1 projection (own half, fp8 DR) ----------------
        qT = big.tile([P, KT, N_OWN], f8, tag="t2m")
        w_sb = wres.tile([P, KT, INNER], f8, tag="wbig")
        nc.sync.dma_start(w_sb[:], wq1_d[:])
        for qcc in range(N_OWN // 256):
            hch = wmed.tile([P, KT, 256], f8, tag="med4")
            nc.sync.dma_start(hch[:], h1T_dram[:, :, qcc * 256:(qcc + 1) * 256])
            for m in range(KT):
                ps = ps_a.tile([P, 512], f32, tag="psa")
                for k2 in range(KT // 2):
                    nc.tensor.matmul(ps[:, 0:256],
                                     w_sb[:, 2 * k2:2 * k2 + 2, m * P:(m + 1) * P],
                                     hch[:, 2 * k2:2 * k2 + 2, :],
                                     start=(k2 == 0), stop=(k2 == KT // 2 - 1),
                                     perf_mode=DRm)
                nc.vector.tensor_scalar_mul(
                    qT[:, m, qcc * 256:(qcc + 1) * 256], ps[:, 0:256], ISC)

        # ---------------- attention (shared for self / cross) ----------------
        def attention(get_k, v_t, qT_t, n_keys_tt, out_T):
            for qc in range(2):
                qsl = slice(qc * 512, (qc + 1) * 512)
                for m2 in range(KT):
                    kap = get_k(m2)
                    hA, hB = 2 * m2, 2 * m2 + 1
                    pavA = ps_av.tile([P, 512], f32, tag="psav")
                    pavB = ps_av.tile([P, 512], f32, tag="psav")
                    exs = {}

                    def av(kt):
                        fl = dict(start=(kt == 0), stop=(kt == n_keys_tt - 1),
                                  skip_group_check=True)
                        ex = exs.pop(kt)
                        nc.tensor.matmul(pavA[0:DH + 1], v_t[:, kt, hA, :],
                                         ex[:, 0, :], **fl)
                        nc.tensor.matmul(pavB[0:DH + 1], v_t[:, kt, hB, :],
                                         ex[:, 1, :], **fl)

                    for kt in range(n_keys_tt):
                        ps = ps_sc.tile([P, 1024], f32, tag="pssc")
                        nc.tensor.matmul(ps[:, 0:512],
                                         kap[0:DH, kt * P:(kt + 1) * P],
                                         qT_t[0:DH, m2, qsl], start=True, stop=True)
                        nc.tensor.matmul(ps[:, 512:1024],
                                         kap[DH:P, kt * P:(kt + 1) * P],
                                         qT_t[DH:P, m2, qsl], start=True, stop=True)
                        ex = expp.tile([P, 2, 512], bf16, tag="expT")
                        nc.scalar.activation(ex[:].rearrange("p a b -> p (a b)"),
                                             ps[:], AF.Exp, scale=SC)
                        exs[kt] = ex
                        if kt > 0:
                            av(kt - 1)
                    av(n_keys_tt - 1)
                    # normalize: den rows -> reciprocal -> PE broadcast -> mult
                    with nc.allow_low_precision(reason="softmax denom"):
                        nc.vector.reciprocal(rec_t[0:1, :], pavA[DH:DH + 1, :])
                        nc.vector.reciprocal(rec_t[DH:DH + 1, :], pavB[DH:DH + 1, :])
                    psD = ps_a.tile([P, 512], f32, tag="psa")
                    nc.tensor.matmul(psD[:], sel65[:], rec_t[0:DH + 1, :],
                                     start=True, stop=True)
                    sD = stats.tile([P, 512], bf16, tag="sD")
                    nc.scalar.copy(sD[:], psD[:])
                    nc.vector.tensor_tensor(out_T[0:DH, m2, qsl],
                                            pavA[0:DH, :], sD[0:DH, :], ALU.mult)
                    nc.vector.tensor_tensor(out_T[DH:P, m2, qsl],
                                            pavB[0:DH, :], sD[DH:P, :], ALU.mult)

        # ---------------- Phase 3: self-attention ----------------
        kth_all = kthp.tile([P, KT, N], f8, tag="kTh", name="kth")
        for m in range(KT):
            nc.sync.dma_start(kth_all[:, m, :], kT_dram[m])

        attn1T = big.tile([P, KT, N_OWN], bf16, tag="t2m")
        attention(lambda m2: kth_all[:, m2, :], v_sb, qT, TT_FULL, attn1T)

        # ---------------- o-proj + residual + LN (fused per tile) ----------
        def out_proj_ln(attn_T, w_sb, bias_idx, res_tiles, n3, dst_sb, dr):
            for tt in range(TT_OWN):
                xt = res_tiles[tt]
                for dch in range(2):
                    ps = ps_a.tile([P, 512], f32, tag="psa")
                    if dr:
                        for m2 in range(KT // 2):
                            nc.tensor.matmul(
                                ps[:], attn_T[:, 2 * m2:2 * m2 + 2, tt * P:(tt + 1) * P],
                                w_sb[:, 2 * m2:2 * m2 + 2, dch * 512:(dch + 1) * 512],
                                start=(m2 == 0), stop=False, perf_mode=DRm,
                                skip_group_check=True)
                    else:
                        for m in range(KT):
                            nc.tensor.matmul(ps[:],
                                             attn_T[:, m, tt * P:(tt + 1) * P],
                                             w_sb[:, m, dch * 512:(dch + 1) * 512],
                                             start=(m == 0), stop=False)
                    nc.tensor.matmul(
                        ps[:], ones1[:],
                        brow_sb[:, bias_idx * D + dch * 512:
                                bias_idx * D + (dch + 1) * 512],
                        start=False, stop=True, skip_group_check=dr)
                    if dr:
                        nc.vector.scalar_tensor_tensor(
                            xt[:, dch * 512:(dch + 1) * 512], ps[:], ISC,
                            xt[:, dch * 512:(dch + 1) * 512], ALU.mult, ALU.add)
                    else:
                        nc.vector.tensor_tensor(
                            xt[:, dch * 512:(dch + 1) * 512], ps[:],
                            xt[:, dch * 512:(dch + 1) * 512], ALU.add)
                layernorm_tile(xt, tt, n3, dst_sb=dst_sb)

        wo1_sb = wres.tile([P, KT, INNER], bf16, tag="wbig")
        nc.sync.dma_start(wo1_sb[:], wo1_d[:])
        h2T = big.tile([P, KT, N_OWN], f8, tag="t2m")
        out_proj_ln(attn1T, wo1_sb, 0, x_own, 1, h2T, dr=False)

        # ---------------- Phase 5: q2 (fp8 DR) ----------------
        q2T = big.tile([P, KT, N_OWN], f8, tag="t2m")
        w_sb = wres.tile([P, KT, INNER], f8, tag="wbig")
        nc.sync.dma_start(w_sb[:], wq2_d[:])
        for m in range(KT):
            for qc2 in range(2):
                ps = ps_a.tile([P, 512], f32, tag="psa")
                for k2 in range(KT // 2):
                    nc.tensor.matmul(ps[:],
                                     w_sb[:, 2 * k2:2 * k2 + 2, m * P:(m + 1) * P],
                                     h2T[:, 2 * k2:2 * k2 + 2, qc2 * 512:(qc2 + 1) * 512],
                                     start=(k2 == 0), stop=(k2 == KT // 2 - 1),
                                     perf_mode=DRm)
                nc.vector.tensor_scalar_mul(q2T[:, m, qc2 * 512:(qc2 + 1) * 512],
                                            ps[:], ISC)

        # ---------------- Phase 6: cross-attention K2/V2 (fp8 DR) ---------
        k2T = kthp.tile([P, KT, J], f8, tag="kTh", name="k2T")
        w_sb = wres.tile([P, CKT, INNER], f8, tag="wbig")
        nc.sync.dma_start(w_sb[:], wk2_d[:])
        for m in range(KT):
            ps = ps_a.tile([P, 512], f32, tag="psa")
            for k2 in range(CKT // 2):
                nc.tensor.matmul(ps[:, 0:J],
                                 w_sb[:, 2 * k2:2 * k2 + 2, m * P:(m + 1) * P],
                                 ctxT_sb[:, 2 * k2:2 * k2 + 2, :],
                                 start=(k2 == 0), stop=(k2 == CKT // 2 - 1),
                                 perf_mode=DRm)
            nc.vector.tensor_scalar_mul(k2T[:, m, :], ps[:, 0:J], ISC)
        v2_sb = vpool.tile([P, J // P, H, DH + 1], bf16, tag="v33", name="v2_sb")
        nc.vector.memset(v2_sb[:, :, :, DH:DH + 1], 1.0)
        w_sb = wres.tile([P, CKT, INNER], f8, tag="wbig")
        nc.sync.dma_start(w_sb[:], wv2_d[:])
        for tt in range(J // P):
            for nc2 in range(2):
                ps = ps_a.tile([P, 512], f32, tag="psa")
                for k2 in range(CKT // 2):
                    nc.tensor.matmul(ps[:],
                                     ctxT_sb[:, 2 * k2:2 * k2 + 2, tt * P:(tt + 1) * P],
                                     w_sb[:, 2 * k2:2 * k2 + 2, nc2 * 512:(nc2 + 1) * 512],
                                     start=(k2 == 0), stop=(k2 == CKT // 2 - 1),
                                     perf_mode=DRm)
                nc.scalar.mul(
                    v2_sb[:, tt, nc2 * 8:(nc2 + 1) * 8, 0:DH],
                    ps[:].rearrange("p (hh r) -> p hh r", r=DH), ISC)

        attn2T = big.tile([P, KT, N_OWN], f8, tag="t2m")
        attention(lambda m2: k2T[:, m2, :], v2_sb, q2T, J // P, attn2T)

        wo2_sb = wres.tile([P, KT, INNER], f8, tag="wbig")
        nc.sync.dma_start(wo2_sb[:], wo2_d[:])
        h3T = big.tile([P, KT, N_OWN], bf16, tag="t2m")
        out_proj_ln(attn2T, wo2_sb, 1, x_own, 2, h3T, dr=True)

        # ---------------- Phase 9: GEGLU FF (bf16) ----------------
        g_sb = big.tile([P, 8, N_OWN], bf16, tag="g_sb", bufs=1)
        for grp in range(4):
            wf2g = wres.tile([P, 8, D], bf16, tag="wbig")
            nc.sync.dma_start(wf2g[:], wf2_d[:, grp * 8:(grp + 1) * 8, :])
            for j in range(8):
                f = grp * 8 + j
                wa = wsm.tile([P, KT, P], bf16, tag="wstream")
                nc.sync.dma_start(wa[:], wf1_d[:, :, f * P:(f + 1) * P])
                wg = wsm.tile([P, KT, P], bf16, tag="wstream")
                nc.sync.dma_start(wg[:], wf1_d[:, :, FF + f * P:FF + (f + 1) * P])
                gt_sb = evict.tile([P, N_OWN], bf16, tag="gt_sb")
                for qc in range(2):
                    sl = slice(qc * 512, (qc + 1) * 512)
                    ps2 = ps_sc.tile([P, 1024], f32, tag="pssc")
                    for kt in range(KT):
                        nc.tensor.matmul(ps2[:, 0:512], wg[:, kt, :], h3T[:, kt, sl],
                                         start=(kt == 0), stop=(kt == KT - 1))
                    nc.scalar.activation(gt_sb[:, sl], ps2[:, 0:512], AF.Gelu,
                                         bias=fb1_sb[:, 32 + f:32 + f + 1])
                    ps1 = ps_sc.tile([P, 1024], f32, tag="pssc")
                    for kt in range(KT):
                        nc.tensor.matmul(ps1[:, 0:512], wa[:, kt, :], h3T[:, kt, sl],
                                         start=(kt == 0), stop=(kt == KT - 1))
                    # g = (a + b1a) * gelu(gate)
                    nc.vector.scalar_tensor_tensor(
                        g_sb[:, j, sl], ps1[:, 0:512], fb1_sb[:, f:f + 1],
                        gt_sb[:, sl], ALU.add, ALU.mult)
            for tt in range(TT_OWN):
                for dc in range(2):
                    sl = slice(dc * 512, (dc + 1) * 512)
                    ps = ps_a.tile([P, 512], f32, tag="psa")
                    for jj in range(8):
                        nc.tensor.matmul(ps[:], g_sb[:, jj, tt * P:(tt + 1) * P],
                                         wf2g[:, jj, sl],
                                         start=(jj == 0),
                                         stop=(jj == 7 and grp != 3))
                    if grp < 3:
                        nc.vector.tensor_tensor(
                            x_own[tt][:, sl], x_own[tt][:, sl], ps[:], ALU.add)
                    else:
                        nc.tensor.matmul(
                            ps[:], ones1[:],
                            brow_sb[:, 2 * D + dc * 512:2 * D + (dc + 1) * 512],
                            start=False, stop=True)
                        yt = evict.tile([P, 512], f32, tag="yt", bufs=1)
                        nc.vector.tensor_tensor(yt[:], x_own[tt][:, sl], ps[:],
                                                ALU.add)
                        nc.sync.dma_start(y_d[tt, :, sl], yt[:])

    nc.compile()
    return nc


def _rearr_w(w, kt):
    return np.ascontiguousarray(
        w.reshape(kt, P, -1).transpose(1, 0, 2)).astype(BF16)


def _rearr_w8(w, kt):
    return np.ascontiguousarray(
        w.reshape(kt, P, -1).transpose(1, 0, 2) * WS).astype(F8E4)


def _shard_inputs(inputs):
    f = {k: np.asarray(v, dtype=np.float32) for k, v in inputs.items()}
    shared = {
        "nw": _rearr_w(np.concatenate([f["n1_w"], f["n2_w"], f["n3_w"]], axis=1), KT),
        "nbc": np.ascontiguousarray(
            np.concatenate([f["n1_b"], f["n2_b"], f["n3_b"]])
            .reshape(3, 16, P).transpose(2, 0, 1).reshape(P, 48)),
        "wq1": _rearr_w8(f["q1"], KT), "wk1": _rearr_w8(f["k1"], KT),
        "wv1": _rearr_w8(f["v1"], KT), "wo1": _rearr_w(f["o1_w"], KT),
        "wq2": _rearr_w8(f["q2"], KT), "wk2": _rearr_w8(f["k2"], CKT),
        "wv2": _rearr_w8(f["v2"], CKT), "wo2": _rearr_w8(f["o2_w"], KT),
        "brow": np.ascontiguousarray(
            np.concatenate([f["o1_b"], WS * f["o2_b"], f["ff_b2"]])[None]).astype(BF16),
        "fb1c": np.ascontiguousarray(f["ff_b1"].reshape(64, P).T),
        "wf1": _rearr_w(f["ff_w1"], KT),
        "wf2": _rearr_w(f["ff_w2"], FF // P),
    }
    in_maps = []
    for core in range(8):
        b, half = core // 2, core % 2
        own = f["x"][b, half * N_OWN:(half + 1) * N_OWN]
        oth = f["x"][b, (1 - half) * N_OWN:(2 - half) * N_OWN]
        m = dict(shared)
        m["xf"] = np.ascontiguousarray(
            np.concatenate([own, oth]).reshape(TT_FULL, P, D))
        m["tT"] = np.ascontiguousarray(f["t"][b, 0].reshape(KT, P).T).astype(BF16)
        m["ctxT"] = np.ascontiguousarray(
            f["context"][b].T.reshape(CKT, P, J).transpose(1, 0, 2)).astype(F8E4)
        in_maps.append(m)
    return in_maps


def kernel(**inputs):
    from concourse.bass_utils import run_bass_kernel_spmd
    if "nc" not in _CACHE:
        _CACHE["nc"] = _build_program()
    nc = _CACHE["nc"]
    in_maps = _shard_inputs(inputs)
    res = run_bass_kernel_spmd(nc, in_maps, core_ids=list(range(8)))
    out = np.empty((B, N, D), dtype=np.float32)
    for core in range(8):
        b, half = core // 2, core % 2
        out[b, half * N_OWN:(half + 1) * N_OWN] = \
            res.results[core]["y"].reshape(N_OWN, D)
    return out


# revision 10
# speedup vs baseline: 1.0691x; 1.0263x over previous
"""Trainium2 Bass kernel for nn_BasicTransformerBlock_35304631173827.

Sharding: 8 cores = 4 samples x 2 sequence halves. Each core computes its
1024-token half of one sample fully locally (self-attention K/V recomputed
over the full 2048-token sample -> zero collectives).

v3: fp8(e4m3) DoubleRow matmuls for Q/K/V/O2 projections (weights pre-scaled
x256 on host, descaled 1/256 at PSUM eviction), attention score matmuls
interleaved across head pairs (PE row-halves 0/64 pipeline concurrently),
softmax scale folded into the exp activation, and a PE-broadcast based
softmax normalization (den rows -> reciprocal -> sel2 matmul broadcast ->
two 64-partition multiplies on vector/gpsimd).
"""

import numpy as np
import ml_dtypes

BF16 = ml_dtypes.bfloat16
F8E4 = ml_dtypes.float8_e4m3

B, N, D = 4, 2048, 1024
J, CD = 256, 768
H, DH = 16, 64
INNER = 1024
FF = 4096
P = 128
KT = D // P            # 8
CKT = CD // P          # 6
TT_FULL = N // P       # 16
N_OWN = N // 2
TT_OWN = N_OWN // P    # 8
EPS = 1e-5
WS = 256.0             # fp8 weight pre-scale (exact power of 2)
ISC = 1.0 / WS

_CACHE = {}


def _build_program():
    import concourse.tile as tile
    from concourse import mybir, bacc
    from concourse.masks import make_identity
    from contextlib import ExitStack

    f32 = mybir.dt.float32
    bf16 = mybir.dt.bfloat16
    f8 = mybir.dt.float8e4
    AF = mybir.ActivationFunctionType
    ALU = mybir.AluOpType
    DRm = mybir.MatmulPerfMode.DoubleRow

    nc = bacc.Bacc(None, target_bir_lowering=False)

    xf_d = nc.dram_tensor("xf", [TT_FULL, P, D], f32, kind="ExternalInput")
    tT_d = nc.dram_tensor("tT", [P, KT], bf16, kind="ExternalInput")
    nw_d = nc.dram_tensor("nw", [P, KT, 6 * D], bf16, kind="ExternalInput")
    nbc_d = nc.dram_tensor("nbc", [P, 48], f32, kind="ExternalInput")
    wq1_d = nc.dram_tensor("wq1", [P, KT, INNER], f8, kind="ExternalInput")
    wk1_d = nc.dram_tensor("wk1", [P, KT, INNER], f8, kind="ExternalInput")
    wv1_d = nc.dram_tensor("wv1", [P, KT, INNER], f8, kind="ExternalInput")
    wo1_d = nc.dram_tensor("wo1", [P, KT, D], bf16, kind="ExternalInput")
    wq2_d = nc.dram_tensor("wq2", [P, KT, INNER], f8, kind="ExternalInput")
    wk2_d = nc.dram_tensor("wk2", [P, CKT, INNER], f8, kind="ExternalInput")
    wv2_d = nc.dram_tensor("wv2", [P, CKT, INNER], f8, kind="ExternalInput")
    wo2_d = nc.dram_tensor("wo2", [P, KT, D], f8, kind="ExternalInput")
    ctxT_d = nc.dram_tensor("ctxT", [P, CKT, J], f8, kind="ExternalInput")
    brow_d = nc.dram_tensor("brow", [1, 3 * D], bf16, kind="ExternalInput")
    fb1_d = nc.dram_tensor("fb1c", [P, 64], f32, kind="ExternalInput")
    wf1_d = nc.dram_tensor("wf1", [P, KT, 2 * FF], bf16, kind="ExternalInput")
    wf2_d = nc.dram_tensor("wf2", [P, FF // P, D], bf16, kind="ExternalInput")
    y_d = nc.dram_tensor("y", [TT_OWN, P, D], f32, kind="ExternalOutput")

    # DRAM scratch: full-sample transposed LN1 output (fp8) and K^T (fp8).
    h1T_dram = nc.dram_tensor("s_h1T", [P, KT, N_OWN], f8, kind="Internal")
    kT_dram = nc.dram_tensor("s_kT", [KT, P, N], f8, kind="Internal")

    SC = DH ** -0.5

    with tile.TileContext(nc) as tc, ExitStack() as es:
        konst = es.enter_context(tc.tile_pool(name="konst", bufs=1))
        xpool = es.enter_context(tc.tile_pool(name="xpool", bufs=1))
        xres = es.enter_context(tc.tile_pool(name="xres", bufs=8))
        stats = es.enter_context(tc.tile_pool(name="stats", bufs=2))
        wres = es.enter_context(tc.tile_pool(name="wres", bufs=2))
        wsm = es.enter_context(tc.tile_pool(name="wsm", bufs=3))
        wmed = es.enter_context(tc.tile_pool(name="wmed", bufs=1))
        evict = es.enter_context(tc.tile_pool(name="evict", bufs=2))
        stg = es.enter_context(tc.tile_pool(name="stg", bufs=2))
        big = es.enter_context(tc.tile_pool(name="big", bufs=2))
        vpool = es.enter_context(tc.tile_pool(name="vpool", bufs=1))
        kthp = es.enter_context(tc.tile_pool(name="kthp", bufs=1))
        expp = es.enter_context(tc.tile_pool(name="expp", bufs=3))
        ps_a = es.enter_context(tc.tile_pool(name="ps_a", bufs=2, space="PSUM"))
        ps_sc = es.enter_context(tc.tile_pool(name="ps_sc", bufs=2, space="PSUM"))
        ps_av = es.enter_context(tc.tile_pool(name="ps_av", bufs=2, space="PSUM"))
        ps_tr = ps_av

        # ---------------- constants ----------------
        ident = konst.tile([P, P], bf16)
        make_identity(nc, ident)
        ones1 = konst.tile([1, P], bf16)
        nc.vector.memset(ones1[:], 1.0)
        eps_t = konst.tile([P, 1], f32)
        nc.vector.memset(eps_t[:], EPS)
        sel65 = konst.tile([DH + 1, P], bf16)
        nc.vector.memset(sel65[:], 0.0)
        nc.vector.memset(sel65[0:1, 0:DH], 1.0)
        nc.vector.memset(sel65[DH:DH + 1, DH:P], 1.0)
        rec_t = konst.tile([P, 512], bf16)
        nc.vector.memset(rec_t[:], 0.0)
        tT_sb = konst.tile([P, KT], bf16)
        nc.sync.dma_start(tT_sb[:], tT_d[:])
        nbc_sb = konst.tile([P, 48], f32)
        nc.sync.dma_start(nbc_sb[:], nbc_d[:])
        fb1_sb = konst.tile([P, 64], f32)
        nc.sync.dma_start(fb1_sb[:], fb1_d[:])
        brow_sb = konst.tile([1, 3 * D], bf16)
        nc.sync.dma_start(brow_sb[:], brow_d[:])
        ctxT_sb = konst.tile([P, CKT, J], f8)
        nc.sync.dma_start(ctxT_sb[:], ctxT_d[:])
        cols = konst.tile([P, 48], f32)

        # ---------------- phase 0 pieces (AdaLN embedding) ----------------
        def p0_chunk(c):
            nwt = wsm.tile([P, KT, P], bf16, tag="wstream")
            nc.sync.dma_start(nwt[:], nw_d[:, :, c * P:(c + 1) * P])
            ps = ps_a.tile([P, 512], f32, tag="psa")
            for kt in range(KT):
                nc.tensor.matmul(ps[:, 0:1], nwt[:, kt, :], tT_sb[:, kt:kt + 1],
                                 start=(kt == 0), stop=(kt == KT - 1))
            nc.vector.tensor_copy(cols[:, c:c + 1], ps[:, 0:1])

        def p0_fixup(n3):
            sl = slice(n3 * 16, (n3 + 1) * 16)
            nc.vector.tensor_add(cols[:, sl], cols[:, sl], nbc_sb[:, sl])
            nc.vector.tensor_scalar_add(cols[:, n3 * 16:n3 * 16 + 8],
                                        cols[:, n3 * 16:n3 * 16 + 8], 1.0)

        def layernorm_tile(x_tile, tt, n3, dst_sb=None, stage=None, soff=0):
            """LayerNorm + AdaLN affine on (P, D) tile -> transposed chunks."""
            bst = stats.tile([P, 2, 6], f32, tag="bnst")
            for g in range(2):
                nc.vector.bn_stats(bst[:, g, :], x_tile[:, g * 512:(g + 1) * 512])
            mv = stats.tile([P, 4], f32, tag="mv")
            nc.vector.bn_aggr(mv[:, 0:2], bst[:])
            nc.scalar.activation(mv[:, 2:3], mv[:, 1:2], AF.Sqrt, bias=eps_t[:])
            nc.vector.reciprocal(mv[:, 2:3], mv[:, 2:3])
            nc.vector.tensor_tensor(mv[:, 3:4], mv[:, 0:1], mv[:, 2:3], ALU.mult)
            nc.vector.tensor_scalar_mul(mv[:, 3:4], mv[:, 3:4], -1.0)
            xn = evict.tile([P, D], bf16, tag="xn")
            nc.scalar.activation(xn[:], x_tile[:], AF.Identity,
                                 bias=mv[:, 3:4], scale=mv[:, 2:3])
            for c in range(KT):
                pt = ps_tr.tile([P, P], bf16, tag="psav")
                nc.tensor.transpose(pt[:], xn[:, c * P:(c + 1) * P], ident[:])
                out_ap = (stage[:, c, soff:soff + P] if stage is not None
                          else dst_sb[:, c, tt * P:(tt + 1) * P])
                nc.vector.tensor_scalar(
                    out_ap, pt[:],
                    cols[:, n3 * 16 + c:n3 * 16 + c + 1],
                    cols[:, n3 * 16 + 8 + c:n3 * 16 + 8 + c + 1],
                    ALU.mult, ALU.add)

        # ------- Phase 0+1+2: LN1 fused with K/V projections -------------
        for c in range(16):
            p0_chunk(c)
        p0_fixup(0)

        wk1_sb = wres.tile([P, KT, INNER], f8, tag="wbig")
        nc.sync.dma_start(wk1_sb[:], wk1_d[:])
        wv1_sb = wres.tile([P, KT, INNER], f8, tag="wbig")
        nc.sync.dma_start(wv1_sb[:], wv1_d[:])

        v_sb = vpool.tile([P, TT_FULL, H, DH + 1], bf16, tag="v33")
        nc.vector.memset(v_sb[:, :, :, DH:DH + 1], 1.0)

        x_own = []
        stage2 = None
        for tt in range(TT_FULL):
            if tt < TT_OWN:
                xt = xres.tile([P, D], f32, tag="xr")
                x_own.append(xt)
            else:
                xt = xpool.tile([P, D], f32, tag="x")
            nc.sync.dma_start(xt[:], xf_d[tt])
            if tt % 2 == 0:
                stage2 = stg.tile([P, KT, 256], f8, tag="stage", name="stage")
            layernorm_tile(xt, tt, 0, stage=stage2, soff=(tt % 2) * P)
            # V projection for this tile straight from the staged LN output
            off = (tt % 2) * P
            for nc2 in range(2):
                ps = ps_a.tile([P, 512], f32, tag="psa")
                for k2 in range(KT // 2):
                    nc.tensor.matmul(ps[:], stage2[:, 2 * k2:2 * k2 + 2, off:off + P],
                                     wv1_sb[:, 2 * k2:2 * k2 + 2,
                                            nc2 * 512:(nc2 + 1) * 512],
                                     start=(k2 == 0), stop=(k2 == KT // 2 - 1),
                                     perf_mode=DRm)
                nc.scalar.mul(
                    v_sb[:, tt, nc2 * 8:(nc2 + 1) * 8, 0:DH],
                    ps[:].rearrange("p (hh r) -> p hh r", r=DH), ISC)
            if tt % 2 == 1:
                c2 = tt // 2
                # K chunk for 256 tokens from the staged pair
                for m in range(KT):
                    ps = ps_a.tile([P, 512], f32, tag="psa")
                    for k2 in range(KT // 2):
                        nc.tensor.matmul(ps[:, 0:256],
                                         wk1_sb[:, 2 * k2:2 * k2 + 2, m * P:(m + 1) * P],
                                         stage2[:, 2 * k2:2 * k2 + 2, :],
                                         start=(k2 == 0), stop=(k2 == KT // 2 - 1),
                                         perf_mode=DRm)
                    kst = stg.tile([P, 256], f8, tag="kstage")
                    nc.vector.tensor_scalar_mul(kst[:], ps[:, 0:256], ISC)
                    nc.sync.dma_start(
                        kT_dram[m, :, c2 * 256:(c2 + 1) * 256], kst[:])
                if tt < TT_OWN:
                    nc.sync.dma_start(
                        h1T_dram[:, :, c2 * 256:(c2 + 1) * 256], stage2[:])
            # stream remaining phase-0 chunks behind LN1 tiles
            for c in range(16 + tt * 2, min(16 + tt * 2 + 2, 48)):
                p0_chunk(c)
            if tt == 7:
                p0_fixup(1)
            if tt == 15:
                p0_fixup(2)

        # ---------------- Q1 projection (own half, fp8 DR) ----------------
        qT = big.tile([P, KT, N_OWN], f8, tag="t2m")
        w_sb = wres.tile([P, KT, INNER], f8, tag="wbig")
        nc.sync.dma_start(w_sb[:], wq1_d[:])
        for qcc in range(N_OWN // 256):
            hch = wmed.tile([P, KT, 256], f8, tag="med4")
            nc.sync.dma_start(hch[:], h1T_dram[:, :, qcc * 256:(qcc + 1) * 256])
            for m in range(KT):
                ps = ps_a.tile([P, 512], f32, tag="psa")
                for k2 in range(KT // 2):
                    nc.tensor.matmul(ps[:, 0:256],
                                     w_sb[:, 2 * k2:2 * k2 + 2, m * P:(m + 1) * P],
                                     hch[:, 2 * k2:2 * k2 + 2, :],
                                     start=(k2 == 0), stop=(k2 == KT // 2 - 1),
                                     perf_mode=DRm)
                nc.vector.tensor_scalar_mul(
                    qT[:, m, qcc * 256:(qcc + 1) * 256], ps[:, 0:256], ISC)

        # ---------------- attention (shared for self / cross) ----------------
        def attention(get_k, v_t, qT_t, n_keys_tt, out_T):
            for qc in range(2):
                qsl = slice(qc * 512, (qc + 1) * 512)
                for m2 in range(KT):
                    kap = get_k(m2)
                    hA, hB = 2 * m2, 2 * m2 + 1
                    pavA = ps_av.tile([P, 512], f32, tag="psav")
                    pavB = ps_av.tile([P, 512], f32, tag="psav")
                    exs = {}

                    def av(kt):
                        fl = dict(start=(kt == 0), stop=(kt == n_keys_tt - 1),
                                  skip_group_check=True)
                        ex = exs.pop(kt)
                        nc.tensor.matmul(pavA[0:DH + 1], v_t[:, kt, hA, :],
                                         ex[:, 0, :], **fl)
                        nc.tensor.matmul(pavB[0:DH + 1], v_t[:, kt, hB, :],
                                         ex[:, 1, :], **fl)

                    for kt in range(n_keys_tt):
                        ps = ps_sc.tile([P, 1024], f32, tag="pssc")
                        nc.tensor.matmul(ps[:, 0:512],
                                         kap[0:DH, kt * P:(kt + 1) * P],
                                         qT_t[0:DH, m2, qsl], start=True, stop=True)
                        nc.tensor.matmul(ps[:, 512:1024],
                                         kap[DH:P, kt * P:(kt + 1) * P],
                                         qT_t[DH:P, m2, qsl], start=True, stop=True)
                        ex = expp.tile([P, 2, 512], bf16, tag="expT")
                        nc.scalar.activation(ex[:].rearrange("p a b -> p (a b)"),
                                             ps[:], AF.Exp, scale=SC)
                        exs[kt] = ex
                        if kt > 0:
                            av(kt - 1)
                    av(n_keys_tt - 1)
                    # normalize: den rows -> PE broadcast -> batched recip -> mult
                    nc.scalar.copy(rec_t[0:1, :], pavA[DH:DH + 1, :])
                    nc.scalar.copy(rec_t[DH:DH + 1, :], pavB[DH:DH + 1, :])
                    psD = ps_a.tile([P, 512], f32, tag="psa")
                    nc.tensor.matmul(psD[:], sel65[:], rec_t[0:DH + 1, :],
                                     start=True, stop=True)
                    sD = stats.tile([P, 512], bf16, tag="sD")
                    with nc.allow_low_precision(reason="softmax denom"):
                        nc.vector.reciprocal(sD[:], psD[:])
                    nc.vector.tensor_tensor(out_T[0:DH, m2, qsl],
                                            pavA[0:DH, :], sD[0:DH, :], ALU.mult)
                    nc.vector.tensor_tensor(out_T[DH:P, m2, qsl],
                                            pavB[0:DH, :], sD[DH:P, :], ALU.mult)

        # ---------------- Phase 3: self-attention ----------------
        kth_all = kthp.tile([P, KT, N], f8, tag="kTh", name="kth")
        for m in range(KT):
            nc.sync.dma_start(kth_all[:, m, :], kT_dram[m])

        attn1T = big.tile([P, KT, N_OWN], bf16, tag="t2m")
        attention(lambda m2: kth_all[:, m2, :], v_sb, qT, TT_FULL, attn1T)

        # ---------------- o-proj + residual + LN (fused per tile) ----------
        def out_proj_ln(attn_T, w_sb, bias_idx, res_tiles, n3, dst_sb, dr):
            for tt in range(TT_OWN):
                xt = res_tiles[tt]
                for dch in range(2):
                    ps = ps_a.tile([P, 512], f32, tag="psa")
                    if dr:
                        for m2 in range(KT // 2):
                            nc.tensor.matmul(
                                ps[:], attn_T[:, 2 * m2:2 * m2 + 2, tt * P:(tt + 1) * P],
                                w_sb[:, 2 * m2:2 * m2 + 2, dch * 512:(dch + 1) * 512],
                                start=(m2 == 0), stop=False, perf_mode=DRm,
                                skip_group_check=True)
                    else:
                        for m in range(KT):
                            nc.tensor.matmul(ps[:],
                                             attn_T[:, m, tt * P:(tt + 1) * P],
                                             w_sb[:, m, dch * 512:(dch + 1) * 512],
                                             start=(m == 0), stop=False)
                    nc.tensor.matmul(
                        ps[:], ones1[:],
                        brow_sb[:, bias_idx * D + dch * 512:
                                bias_idx * D + (dch + 1) * 512],
                        start=False, stop=True, skip_group_check=dr)
                    if dr:
                        nc.vector.scalar_tensor_tensor(
                            xt[:, dch * 512:(dch + 1) * 512], ps[:], ISC,
                            xt[:, dch * 512:(dch + 1) * 512], ALU.mult, ALU.add)
                    else:
                        nc.vector.tensor_tensor(
                            xt[:, dch * 512:(dch + 1) * 512], ps[:],
                            xt[:, dch * 512:(dch + 1) * 512], ALU.add)
                layernorm_tile(xt, tt, n3, dst_sb=dst_sb)

        wo1_sb = wres.tile([P, KT, INNER], bf16, tag="wbig")
        nc.sync.dma_start(wo1_sb[:], wo1_d[:])
        h2T = big.tile([P, KT, N_OWN], f8, tag="t2m")
        out_proj_ln(attn1T, wo1_sb, 0, x_own, 1, h2T, dr=False)

        # ---------------- Phase 5: q2 (fp8 DR) ----------------
        q2T = big.tile([P, KT, N_OWN], f8, tag="t2m")
        w_sb = wres.tile([P, KT, INNER], f8, tag="wbig")
        nc.sync.dma_start(w_sb[:], wq2_d[:])
        for m in range(KT):
            for qc2 in range(2):
                ps = ps_a.tile([P, 512], f32, tag="psa")
                for k2 in range(KT // 2):
                    nc.tensor.matmul(ps[:],
                                     w_sb[:, 2 * k2:2 * k2 + 2, m * P:(m + 1) * P],
                                     h2T[:, 2 * k2:2 * k2 + 2, qc2 * 512:(qc2 + 1) * 512],
                                     start=(k2 == 0), stop=(k2 == KT // 2 - 1),
                                     perf_mode=DRm)
                nc.vector.tensor_scalar_mul(q2T[:, m, qc2 * 512:(qc2 + 1) * 512],
                                            ps[:], ISC)

        # ---------------- Phase 6: cross-attention K2/V2 (fp8 DR) ---------
        k2T = kthp.tile([P, KT, J], f8, tag="kTh", name="k2T")
        w_sb = wres.tile([P, CKT, INNER], f8, tag="wbig")
        nc.sync.dma_start(w_sb[:], wk2_d[:])
        for m in range(KT):
            ps = ps_a.tile([P, 512], f32, tag="psa")
            for k2 in range(CKT // 2):
                nc.tensor.matmul(ps[:, 0:J],
                                 w_sb[:, 2 * k2:2 * k2 + 2, m * P:(m + 1) * P],
                                 ctxT_sb[:, 2 * k2:2 * k2 + 2, :],
                                 start=(k2 == 0), stop=(k2 == CKT // 2 - 1),
                                 perf_mode=DRm)
            nc.vector.tensor_scalar_mul(k2T[:, m, :], ps[:, 0:J], ISC)
        v2_sb = vpool.tile([P, J // P, H, DH + 1], bf16, tag="v33", name="v2_sb")
        nc.vector.memset(v2_sb[:, :, :, DH:DH + 1], 1.0)
        w_sb = wres.tile([P, CKT, INNER], f8, tag="wbig")
        nc.sync.dma_start(w_sb[:], wv2_d[:])
        for tt in range(J // P):
            for nc2 in range(2):
                ps = ps_a.tile([P, 512], f32, tag="psa")
                for k2 in range(CKT // 2):
                    nc.tensor.matmul(ps[:],
                                     ctxT_sb[:, 2 * k2:2 * k2 + 2, tt * P:(tt + 1) * P],
                                     w_sb[:, 2 * k2:2 * k2 + 2, nc2 * 512:(nc2 + 1) * 512],
                                     start=(k2 == 0), stop=(k2 == CKT // 2 - 1),
                                     perf_mode=DRm)
                nc.scalar.mul(
                    v2_sb[:, tt, nc2 * 8:(nc2 + 1) * 8, 0:DH],
                    ps[:].rearrange("p (hh r) -> p hh r", r=DH), ISC)

        attn2T = big.tile([P, KT, N_OWN], f8, tag="t2m")
        attention(lambda m2: k2T[:, m2, :], v2_sb, q2T, J // P, attn2T)

        wo2_sb = wres.tile([P, KT, INNER], f8, tag="wbig")
        nc.sync.dma_start(wo2_sb[:], wo2_d[:])
        h3T = big.tile([P, KT, N_OWN], bf16, tag="t2m")
        out_proj_ln(attn2T, wo2_sb, 1, x_own, 2, h3T, dr=True)

        # ---------------- Phase 9: GEGLU FF (bf16) ----------------
        g_sb = big.tile([P, 8, N_OWN], bf16, tag="g_sb", bufs=1)
        for grp in range(4):
            wf2g = wres.tile([P, 8, D], bf16, tag="wbig")
            nc.sync.dma_start(wf2g[:], wf2_d[:, grp * 8:(grp + 1) * 8, :])
            for j in range(8):
                f = grp * 8 + j
                wa = wsm.tile([P, KT, P], bf16, tag="wstream")
                nc.sync.dma_start(wa[:], wf1_d[:, :, f * P:(f + 1) * P])
                wg = wsm.tile([P, KT, P], bf16, tag="wstream")
                nc.sync.dma_start(wg[:], wf1_d[:, :, FF + f * P:FF + (f + 1) * P])
                gt_sb = evict.tile([P, N_OWN], bf16, tag="gt_sb")
                for qc in range(2):
                    sl = slice(qc * 512, (qc + 1) * 512)
                    ps2 = ps_sc.tile([P, 1024], f32, tag="pssc")
                    for kt in range(KT):
                        nc.tensor.matmul(ps2[:, 0:512], wg[:, kt, :], h3T[:, kt, sl],
                                         start=(kt == 0), stop=(kt == KT - 1))
                    nc.scalar.activation(gt_sb[:, sl], ps2[:, 0:512], AF.Gelu,
                                         bias=fb1_sb[:, 32 + f:32 + f + 1])
                    ps1 = ps_sc.tile([P, 1024], f32, tag="pssc")
                    for kt in range(KT):
                        nc.tensor.matmul(ps1[:, 0:512], wa[:, kt, :], h3T[:, kt, sl],
                                         start=(kt == 0), stop=(kt == KT - 1))
                    # g = (a + b1a) * gelu(gate)
                    nc.vector.scalar_tensor_tensor(
                        g_sb[:, j, sl], ps1[:, 0:512], fb1_sb[:, f:f + 1],
                        gt_sb[:, sl], ALU.add, ALU.mult)
            for tt in range(TT_OWN):
                for dc in range(2):
                    sl = slice(dc * 512, (dc + 1) * 512)
                    ps = ps_a.tile([P, 512], f32, tag="psa")
                    for jj in range(8):
                        nc.tensor.matmul(ps[:], g_sb[:, jj, tt * P:(tt + 1) * P],
                                         wf2g[:, jj, sl],
                                         start=(jj == 0),
                                         stop=(jj == 7 and grp != 3))
                    if grp < 3:
                        nc.vector.tensor_tensor(
                            x_own[tt][:, sl], x_own[tt][:, sl], ps[:], ALU.add)
                    else:
                        nc.tensor.matmul(
                            ps[:], ones1[:],
                            brow_sb[:, 2 * D + dc * 512:2 * D + (dc + 1) * 512],
                            start=False, stop=True)
                        yt = evict.tile([P, 512], f32, tag="yt", bufs=1)
                        nc.vector.tensor_tensor(yt[:], x_own[tt][:, sl], ps[:],
                                                ALU.add)
                        nc.sync.dma_start(y_d[tt, :, sl], yt[:])

    nc.compile()
    return nc


def _rearr_w(w, kt):
    return np.ascontiguousarray(
        w.reshape(kt, P, -1).transpose(1, 0, 2)).astype(BF16)


def _rearr_w8(w, kt):
    return np.ascontiguousarray(
        w.reshape(kt, P, -1).transpose(1, 0, 2) * WS).astype(F8E4)


def _shard_inputs(inputs):
    f = {k: np.asarray(v, dtype=np.float32) for k, v in inputs.items()}
    shared = {
        "nw": _rearr_w(np.concatenate([f["n1_w"], f["n2_w"], f["n3_w"]], axis=1), KT),
        "nbc": np.ascontiguousarray(
            np.concatenate([f["n1_b"], f["n2_b"], f["n3_b"]])
            .reshape(3, 16, P).transpose(2, 0, 1).reshape(P, 48)),
        "wq1": _rearr_w8(f["q1"], KT), "wk1": _rearr_w8(f["k1"], KT),
        "wv1": _rearr_w8(f["v1"], KT), "wo1": _rearr_w(f["o1_w"], KT),
        "wq2": _rearr_w8(f["q2"], KT), "wk2": _rearr_w8(f["k2"], CKT),
        "wv2": _rearr_w8(f["v2"], CKT), "wo2": _rearr_w8(f["o2_w"], KT),
        "brow": np.ascontiguousarray(
            np.concatenate([f["o1_b"], WS * f["o2_b"], f["ff_b2"]])[None]).astype(BF16),
        "fb1c": np.ascontiguousarray(f["ff_b1"].reshape(64, P).T),
        "wf1": _rearr_w(f["ff_w1"], KT),
        "wf2": _rearr_w(f["ff_w2"], FF // P),
    }
    in_maps = []
    for core in range(8):
        b, half = core // 2, core % 2
        own = f["x"][b, half * N_OWN:(half + 1) * N_OWN]
        oth = f["x"][b, (1 - half) * N_OWN:(2 - half) * N_OWN]
        m = dict(shared)
        m["xf"] = np.ascontiguousarray(
            np.concatenate([own, oth]).reshape(TT_FULL, P, D))
        m["tT"] = np.ascontiguousarray(f["t"][b, 0].reshape(KT, P).T).astype(BF16)
        m["ctxT"] = np.ascontiguousarray(
            f["context"][b].T.reshape(CKT, P, J).transpose(1, 0, 2)).astype(F8E4)
        in_maps.append(m)
    return in_maps


def kernel(**inputs):
    from concourse.bass_utils import run_bass_kernel_spmd
    if "nc" not in _CACHE:
        _CACHE["nc"] = _build_program()
    nc = _CACHE["nc"]
    in_maps = _shard_inputs(inputs)
    res = run_bass_kernel_spmd(nc, in_maps, core_ids=list(range(8)))
    out = np.empty((B, N, D), dtype=np.float32)
    for core in range(8):
        b, half = core // 2, core % 2
        out[b, half * N_OWN:(half + 1) * N_OWN] = \
            res.results[core]["y"].reshape(N_OWN, D)
    return out


# revision 15
# speedup vs baseline: 1.1675x; 1.0920x over previous
"""Trainium2 Bass kernel for nn_BasicTransformerBlock_35304631173827.

Sharding: 8 cores = 4 samples x 2 sequence halves. Each core computes its
1024-token half of one sample fully locally (self-attention K/V recomputed
over the full 2048-token sample -> zero collectives).

v3: fp8(e4m3) DoubleRow matmuls for Q/K/V/O2 projections (weights pre-scaled
x256 on host, descaled 1/256 at PSUM eviction), attention score matmuls
interleaved across head pairs (PE row-halves 0/64 pipeline concurrently),
softmax scale folded into the exp activation, and a PE-broadcast based
softmax normalization (den rows -> reciprocal -> sel2 matmul broadcast ->
two 64-partition multiplies on vector/gpsimd).
"""

import numpy as np
import ml_dtypes

BF16 = ml_dtypes.bfloat16
F8E4 = ml_dtypes.float8_e4m3

B, N, D = 4, 2048, 1024
J, CD = 256, 768
H, DH = 16, 64
INNER = 1024
FF = 4096
P = 128
KT = D // P            # 8
CKT = CD // P          # 6
TT_FULL = N // P       # 16
N_OWN = N // 2
TT_OWN = N_OWN // P    # 8
EPS = 1e-5
WS = 256.0             # fp8 weight pre-scale (exact power of 2)
ISC = 1.0 / WS

_CACHE = {}


def _build_program():
    import concourse.tile as tile
    from concourse import mybir, bacc
    from concourse.masks import make_identity
    from contextlib import ExitStack

    f32 = mybir.dt.float32
    bf16 = mybir.dt.bfloat16
    f8 = mybir.dt.float8e4
    AF = mybir.ActivationFunctionType
    ALU = mybir.AluOpType
    DRm = mybir.MatmulPerfMode.DoubleRow

    nc = bacc.Bacc(None, target_bir_lowering=False)

    xf_d = nc.dram_tensor("xf", [TT_FULL, P, D], f32, kind="ExternalInput")
    tT_d = nc.dram_tensor("tT", [P, KT], bf16, kind="ExternalInput")
    nw_d = nc.dram_tensor("nw", [P, KT, 6 * D], bf16, kind="ExternalInput")
    nbc_d = nc.dram_tensor("nbc", [P, 48], f32, kind="ExternalInput")
    wq1_d = nc.dram_tensor("wq1", [P, KT, INNER], f8, kind="ExternalInput")
    wk1_d = nc.dram_tensor("wk1", [P, KT, INNER], f8, kind="ExternalInput")
    wv1_d = nc.dram_tensor("wv1", [P, KT, INNER], f8, kind="ExternalInput")
    wo1_d = nc.dram_tensor("wo1", [P, KT, D], bf16, kind="ExternalInput")
    wq2_d = nc.dram_tensor("wq2", [P, KT, INNER], f8, kind="ExternalInput")
    wk2_d = nc.dram_tensor("wk2", [P, CKT, INNER], f8, kind="ExternalInput")
    wv2_d = nc.dram_tensor("wv2", [P, CKT, INNER], f8, kind="ExternalInput")
    wo2_d = nc.dram_tensor("wo2", [P, KT, D], f8, kind="ExternalInput")
    ctxT_d = nc.dram_tensor("ctxT", [P, CKT, J], f8, kind="ExternalInput")
    brow_d = nc.dram_tensor("brow", [1, 3 * D], bf16, kind="ExternalInput")
    fb1_d = nc.dram_tensor("fb1c", [P, 64], f32, kind="ExternalInput")
    wf1_d = nc.dram_tensor("wf1", [P, KT, 2 * FF], bf16, kind="ExternalInput")
    wf2_d = nc.dram_tensor("wf2", [P, FF // P, D], bf16, kind="ExternalInput")
    y_d = nc.dram_tensor("y", [TT_OWN, P, D], f32, kind="ExternalOutput")

    # DRAM scratch: full-sample transposed LN1 output (fp8) and K^T (fp8).
    h1T_dram = nc.dram_tensor("s_h1T", [P, KT, N_OWN], f8, kind="Internal")
    kT_dram = nc.dram_tensor("s_kT", [KT, P, N], f8, kind="Internal")

    SC = DH ** -0.5

    with tile.TileContext(nc) as tc, ExitStack() as es:
        konst = es.enter_context(tc.tile_pool(name="konst", bufs=1))
        xpool = es.enter_context(tc.tile_pool(name="xpool", bufs=1))
        xres = es.enter_context(tc.tile_pool(name="xres", bufs=8))
        stats = es.enter_context(tc.tile_pool(name="stats", bufs=2))
        wres = es.enter_context(tc.tile_pool(name="wres", bufs=2))
        wsm = es.enter_context(tc.tile_pool(name="wsm", bufs=3))
        wmed = es.enter_context(tc.tile_pool(name="wmed", bufs=1))
        evict = es.enter_context(tc.tile_pool(name="evict", bufs=2))
        stg = es.enter_context(tc.tile_pool(name="stg", bufs=2))
        big = es.enter_context(tc.tile_pool(name="big", bufs=2))
        vpool = es.enter_context(tc.tile_pool(name="vpool", bufs=1))
        kthp = es.enter_context(tc.tile_pool(name="kthp", bufs=1))
        expp = es.enter_context(tc.tile_pool(name="expp", bufs=3))
        ps_a = es.enter_context(tc.tile_pool(name="ps_a", bufs=2, space="PSUM"))
        ps_sc = es.enter_context(tc.tile_pool(name="ps_sc", bufs=2, space="PSUM"))
        ps_av = es.enter_context(tc.tile_pool(name="ps_av", bufs=2, space="PSUM"))
        ps_tr = ps_av

        # ---------------- constants ----------------
        ident = konst.tile([P, P], bf16)
        make_identity(nc, ident)
        ones1 = konst.tile([1, P], bf16)
        nc.vector.memset(ones1[:], 1.0)
        eps_t = konst.tile([P, 1], f32)
        nc.vector.memset(eps_t[:], EPS)
        sel65 = konst.tile([DH + 1, P], bf16)
        nc.vector.memset(sel65[:], 0.0)
        nc.vector.memset(sel65[0:1, 0:DH], 1.0)
        nc.vector.memset(sel65[DH:DH + 1, DH:P], 1.0)
        rec_t = konst.tile([P, 512], bf16)
        nc.vector.memset(rec_t[:], 0.0)
        tT_sb = konst.tile([P, KT], bf16)
        nc.sync.dma_start(tT_sb[:], tT_d[:])
        nbc_sb = konst.tile([P, 48], f32)
        nc.sync.dma_start(nbc_sb[:], nbc_d[:])
        fb1_sb = konst.tile([P, 64], f32)
        nc.sync.dma_start(fb1_sb[:], fb1_d[:])
        brow_sb = konst.tile([1, 3 * D], bf16)
        nc.sync.dma_start(brow_sb[:], brow_d[:])
        ctxT_sb = konst.tile([P, CKT, J], f8)
        nc.sync.dma_start(ctxT_sb[:], ctxT_d[:])
        cols = konst.tile([P, 48], f32)

        # ---------------- phase 0 pieces (AdaLN embedding) ----------------
        def p0_chunk(c):
            nwt = wsm.tile([P, KT, P], bf16, tag="wstream")
            nc.sync.dma_start(nwt[:], nw_d[:, :, c * P:(c + 1) * P])
            ps = ps_a.tile([P, 512], f32, tag="psa")
            for kt in range(KT):
                nc.tensor.matmul(ps[:, 0:1], nwt[:, kt, :], tT_sb[:, kt:kt + 1],
                                 start=(kt == 0), stop=(kt == KT - 1))
            nc.vector.tensor_copy(cols[:, c:c + 1], ps[:, 0:1])

        def p0_fixup(n3):
            sl = slice(n3 * 16, (n3 + 1) * 16)
            nc.vector.tensor_add(cols[:, sl], cols[:, sl], nbc_sb[:, sl])
            nc.vector.tensor_scalar_add(cols[:, n3 * 16:n3 * 16 + 8],
                                        cols[:, n3 * 16:n3 * 16 + 8], 1.0)

        def layernorm_tile(x_tile, tt, n3, dst_sb=None, stage=None, soff=0,
                           n_scalar_evicts=4):
            """LayerNorm + AdaLN affine on (P, D) tile -> transposed chunks."""
            bst = stats.tile([P, 2, 6], f32, tag="bnst")
            for g in range(2):
                nc.vector.bn_stats(bst[:, g, :], x_tile[:, g * 512:(g + 1) * 512])
            mv = stats.tile([P, 4], f32, tag="mv")
            nc.vector.bn_aggr(mv[:, 0:2], bst[:])
            nc.scalar.activation(mv[:, 2:3], mv[:, 1:2], AF.Sqrt, bias=eps_t[:])
            nc.vector.reciprocal(mv[:, 2:3], mv[:, 2:3])
            nc.vector.tensor_tensor(mv[:, 3:4], mv[:, 0:1], mv[:, 2:3], ALU.mult)
            nc.vector.tensor_scalar_mul(mv[:, 3:4], mv[:, 3:4], -1.0)
            xn = evict.tile([P, D], bf16, tag="xn")
            nc.scalar.activation(xn[:], x_tile[:], AF.Identity,
                                 bias=mv[:, 3:4], scale=mv[:, 2:3])
            for c in range(KT):
                pt = ps_tr.tile([P, P], bf16, tag="psav")
                nc.tensor.transpose(pt[:], xn[:, c * P:(c + 1) * P], ident[:])
                out_ap = (stage[:, c, soff:soff + P] if stage is not None
                          else dst_sb[:, c, tt * P:(tt + 1) * P])
                sc_ap = cols[:, n3 * 16 + c:n3 * 16 + c + 1]
                sh_ap = cols[:, n3 * 16 + 8 + c:n3 * 16 + 8 + c + 1]
                if c < n_scalar_evicts:
                    nc.scalar.activation(out_ap, pt[:], AF.Identity,
                                         bias=sh_ap, scale=sc_ap)
                else:
                    nc.vector.tensor_scalar(out_ap, pt[:], sc_ap, sh_ap,
                                            ALU.mult, ALU.add)

        # ------- Phase 0+1+2: LN1 fused with K/V projections -------------
        for c in range(16):
            p0_chunk(c)
        p0_fixup(0)

        wk1_sb = wres.tile([P, KT, INNER], f8, tag="wbig")
        nc.sync.dma_start(wk1_sb[:], wk1_d[:])
        wv1_sb = wres.tile([P, KT, INNER], f8, tag="wbig")
        nc.sync.dma_start(wv1_sb[:], wv1_d[:])

        v_sb = vpool.tile([P, TT_FULL, H, DH + 1], bf16, tag="v33")
        nc.vector.memset(v_sb[:, :, :, DH:DH + 1], 1.0)

        x_own = []
        stage2 = None
        for tt in range(TT_FULL):
            if tt < TT_OWN:
                xt = xres.tile([P, D], f32, tag="xr")
                x_own.append(xt)
            else:
                xt = xpool.tile([P, D], f32, tag="x")
            nc.sync.dma_start(xt[:], xf_d[tt])
            if tt % 2 == 0:
                stage2 = stg.tile([P, KT, 256], f8, tag="stage", name="stage")
            layernorm_tile(xt, tt, 0, stage=stage2, soff=(tt % 2) * P)
            # V projection for this tile straight from the staged LN output
            off = (tt % 2) * P
            for nc2 in range(2):
                ps = ps_a.tile([P, 512], f32, tag="psa")
                for k2 in range(KT // 2):
                    nc.tensor.matmul(ps[:], stage2[:, 2 * k2:2 * k2 + 2, off:off + P],
                                     wv1_sb[:, 2 * k2:2 * k2 + 2,
                                            nc2 * 512:(nc2 + 1) * 512],
                                     start=(k2 == 0), stop=(k2 == KT // 2 - 1),
                                     perf_mode=DRm)
                nc.scalar.mul(
                    v_sb[:, tt, nc2 * 8:(nc2 + 1) * 8, 0:DH],
                    ps[:].rearrange("p (hh r) -> p hh r", r=DH), ISC)
            if tt % 2 == 1:
                c2 = tt // 2
                # K chunk for 256 tokens from the staged pair
                for m in range(KT):
                    ps = ps_a.tile([P, 512], f32, tag="psa")
                    for k2 in range(KT // 2):
                        nc.tensor.matmul(ps[:, 0:256],
                                         wk1_sb[:, 2 * k2:2 * k2 + 2, m * P:(m + 1) * P],
                                         stage2[:, 2 * k2:2 * k2 + 2, :],
                                         start=(k2 == 0), stop=(k2 == KT // 2 - 1),
                                         perf_mode=DRm)
                    kst = stg.tile([P, 256], f8, tag="kstage")
                    nc.vector.tensor_scalar_mul(kst[:], ps[:, 0:256], ISC)
                    nc.sync.dma_start(
                        kT_dram[m, :, c2 * 256:(c2 + 1) * 256], kst[:])
                if tt < TT_OWN:
                    nc.sync.dma_start(
                        h1T_dram[:, :, c2 * 256:(c2 + 1) * 256], stage2[:])
            # stream remaining phase-0 chunks behind LN1 tiles
            for c in range(16 + tt * 2, min(16 + tt * 2 + 2, 48)):
                p0_chunk(c)
            if tt == 7:
                p0_fixup(1)
            if tt == 15:
                p0_fixup(2)

        # ---------------- Q1 projection (own half, fp8 DR) ----------------
        qT = big.tile([P, KT, N_OWN], f8, tag="t2m")
        w_sb = wres.tile([P, KT, INNER], f8, tag="wbig")
        nc.sync.dma_start(w_sb[:], wq1_d[:])
        for qcc in range(N_OWN // 256):
            hch = wmed.tile([P, KT, 256], f8, tag="med4")
            nc.sync.dma_start(hch[:], h1T_dram[:, :, qcc * 256:(qcc + 1) * 256])
            for m in range(KT):
                ps = ps_a.tile([P, 512], f32, tag="psa")
                for k2 in range(KT // 2):
                    nc.tensor.matmul(ps[:, 0:256],
                                     w_sb[:, 2 * k2:2 * k2 + 2, m * P:(m + 1) * P],
                                     hch[:, 2 * k2:2 * k2 + 2, :],
                                     start=(k2 == 0), stop=(k2 == KT // 2 - 1),
                                     perf_mode=DRm)
                nc.vector.tensor_scalar_mul(
                    qT[:, m, qcc * 256:(qcc + 1) * 256], ps[:, 0:256], ISC)

        # ---------------- attention (shared for self / cross) ----------------
        def attention(get_k, v_t, qT_t, n_keys_tt, out_T, fillers=()):
            fill = list(fillers)
            fi = 0
            for qc in range(2):
                qsl = slice(qc * 512, (qc + 1) * 512)
                for m2 in range(KT):
                    kap = get_k(m2)
                    hA, hB = 2 * m2, 2 * m2 + 1
                    pavA = ps_av.tile([P, 512], f32, tag="psav")
                    pavB = ps_av.tile([P, 512], f32, tag="psav")
                    exs = {}

                    def av(kt):
                        fl = dict(start=(kt == 0), stop=(kt == n_keys_tt - 1),
                                  skip_group_check=True)
                        ex = exs.pop(kt)
                        nc.tensor.matmul(pavA[0:DH + 1], v_t[:, kt, hA, :],
                                         ex[:, 0, :], **fl)
                        nc.tensor.matmul(pavB[0:DH + 1], v_t[:, kt, hB, :],
                                         ex[:, 1, :], **fl)

                    for kt in range(n_keys_tt):
                        ps = ps_sc.tile([P, 1024], f32, tag="pssc")
                        nc.tensor.matmul(ps[:, 0:512],
                                         kap[0:DH, kt * P:(kt + 1) * P],
                                         qT_t[0:DH, m2, qsl], start=True, stop=True)
                        nc.tensor.matmul(ps[:, 512:1024],
                                         kap[DH:P, kt * P:(kt + 1) * P],
                                         qT_t[DH:P, m2, qsl], start=True, stop=True)
                        ex = expp.tile([P, 2, 512], bf16, tag="expT")
                        nc.scalar.activation(ex[:].rearrange("p a b -> p (a b)"),
                                             ps[:], AF.Exp, scale=SC)
                        exs[kt] = ex
                        if kt > 0:
                            av(kt - 1)
                    av(n_keys_tt - 1)
                    # normalize: evict pav early (frees PSUM for the next
                    # pair), PE-broadcast the den rows, one batched
                    # reciprocal, one full-width multiply.
                    un = stats.tile([P, 512], bf16, tag="un")
                    nc.vector.tensor_copy(un[0:DH, :], pavA[0:DH, :])
                    nc.vector.tensor_copy(un[DH:P, :], pavB[0:DH, :])
                    nc.scalar.copy(rec_t[0:1, :], pavA[DH:DH + 1, :])
                    nc.scalar.copy(rec_t[DH:DH + 1, :], pavB[DH:DH + 1, :])
                    psD = ps_a.tile([P, 512], f32, tag="psa")
                    nc.tensor.matmul(psD[:], sel65[:], rec_t[0:DH + 1, :],
                                     start=True, stop=True)
                    sD = stats.tile([P, 512], bf16, tag="sD")
                    with nc.allow_low_precision(reason="softmax denom"):
                        nc.vector.reciprocal(sD[:], psD[:])
                    nc.vector.tensor_tensor(out_T[:, m2, qsl], un[:], sD[:],
                                            ALU.mult)
                    if qc == 1 and m2 % 2 == 1 and fi < len(fill):
                        fill[fi]()
                        fi += 1
            while fi < len(fill):
                fill[fi]()
                fi += 1

        # ---------------- o-proj + residual (per tile) ----------------
        def o_proj_tile(attn_T, w_sb, bias_idx, xt, tt, dr):
            for dch in range(2):
                ps = ps_a.tile([P, 512], f32, tag="psa")
                if dr:
                    for m2 in range(KT // 2):
                        nc.tensor.matmul(
                            ps[:], attn_T[:, 2 * m2:2 * m2 + 2, tt * P:(tt + 1) * P],
                            w_sb[:, 2 * m2:2 * m2 + 2, dch * 512:(dch + 1) * 512],
                            start=(m2 == 0), stop=False, perf_mode=DRm,
                            skip_group_check=True)
                else:
                    for m in range(KT):
                        nc.tensor.matmul(ps[:],
                                         attn_T[:, m, tt * P:(tt + 1) * P],
                                         w_sb[:, m, dch * 512:(dch + 1) * 512],
                                         start=(m == 0), stop=False)
                nc.tensor.matmul(
                    ps[:], ones1[:],
                    brow_sb[:, bias_idx * D + dch * 512:
                            bias_idx * D + (dch + 1) * 512],
                    start=False, stop=True, skip_group_check=dr)
                if dr:
                    nc.vector.scalar_tensor_tensor(
                        xt[:, dch * 512:(dch + 1) * 512], ps[:], ISC,
                        xt[:, dch * 512:(dch + 1) * 512], ALU.mult, ALU.add)
                else:
                    nc.vector.tensor_tensor(
                        xt[:, dch * 512:(dch + 1) * 512], ps[:],
                        xt[:, dch * 512:(dch + 1) * 512], ALU.add)

        # ---------------- Phase 3: self-attention ----------------
        kth_all = kthp.tile([P, KT, N], f8, tag="kTh", name="kth")
        for m in range(KT):
            nc.sync.dma_start(kth_all[:, m, :], kT_dram[m])

        wo1_sb = wres.tile([P, KT, INNER], bf16, tag="wbig")
        nc.sync.dma_start(wo1_sb[:], wo1_d[:])
        attn1T = big.tile([P, KT, N_OWN], bf16, tag="t2m")
        # o-proj for the first query half interleaved into the second half's
        # exp-bound attention stream.
        o1_fill = [
            (lambda tt=tt: o_proj_tile(attn1T, wo1_sb, 0, x_own[tt], tt, False))
            for tt in range(4)]
        attention(lambda m2: kth_all[:, m2, :], v_sb, qT, TT_FULL, attn1T,
                  fillers=o1_fill)
        for tt in range(4, TT_OWN):
            o_proj_tile(attn1T, wo1_sb, 0, x_own[tt], tt, False)
        h2T = big.tile([P, KT, N_OWN], f8, tag="t2m")
        for tt in range(TT_OWN):
            layernorm_tile(x_own[tt], tt, 1, dst_sb=h2T)

        # ---------------- Phase 5: q2 (fp8 DR) ----------------
        q2T = big.tile([P, KT, N_OWN], f8, tag="t2m")
        w_sb = wres.tile([P, KT, INNER], f8, tag="wbig")
        nc.sync.dma_start(w_sb[:], wq2_d[:])
        for m in range(KT):
            for qc2 in range(2):
                ps = ps_a.tile([P, 512], f32, tag="psa")
                for k2 in range(KT // 2):
                    nc.tensor.matmul(ps[:],
                                     w_sb[:, 2 * k2:2 * k2 + 2, m * P:(m + 1) * P],
                                     h2T[:, 2 * k2:2 * k2 + 2, qc2 * 512:(qc2 + 1) * 512],
                                     start=(k2 == 0), stop=(k2 == KT // 2 - 1),
                                     perf_mode=DRm)
                nc.vector.tensor_scalar_mul(q2T[:, m, qc2 * 512:(qc2 + 1) * 512],
                                            ps[:], ISC)

        # ---------------- Phase 6: cross-attention K2/V2 (fp8 DR) ---------
        k2T = kthp.tile([P, KT, J], f8, tag="kTh", name="k2T")
        w_sb = wres.tile([P, CKT, INNER], f8, tag="wbig")
        nc.sync.dma_start(w_sb[:], wk2_d[:])
        for m in range(KT):
            ps = ps_a.tile([P, 512], f32, tag="psa")
            for k2 in range(CKT // 2):
                nc.tensor.matmul(ps[:, 0:J],
                                 w_sb[:, 2 * k2:2 * k2 + 2, m * P:(m + 1) * P],
                                 ctxT_sb[:, 2 * k2:2 * k2 + 2, :],
                                 start=(k2 == 0), stop=(k2 == CKT // 2 - 1),
                                 perf_mode=DRm)
            nc.vector.tensor_scalar_mul(k2T[:, m, :], ps[:, 0:J], ISC)
        v2_sb = vpool.tile([P, J // P, H, DH + 1], bf16, tag="v33", name="v2_sb")
        nc.vector.memset(v2_sb[:, :, :, DH:DH + 1], 1.0)
        w_sb = wres.tile([P, CKT, INNER], f8, tag="wbig")
        nc.sync.dma_start(w_sb[:], wv2_d[:])
        for tt in range(J // P):
            for nc2 in range(2):
                ps = ps_a.tile([P, 512], f32, tag="psa")
                for k2 in range(CKT // 2):
                    nc.tensor.matmul(ps[:],
                                     ctxT_sb[:, 2 * k2:2 * k2 + 2, tt * P:(tt + 1) * P],
                                     w_sb[:, 2 * k2:2 * k2 + 2, nc2 * 512:(nc2 + 1) * 512],
                                     start=(k2 == 0), stop=(k2 == CKT // 2 - 1),
                                     perf_mode=DRm)
                nc.scalar.mul(
                    v2_sb[:, tt, nc2 * 8:(nc2 + 1) * 8, 0:DH],
                    ps[:].rearrange("p (hh r) -> p hh r", r=DH), ISC)

        attn2T = big.tile([P, KT, N_OWN], f8, tag="t2m")
        attention(lambda m2: k2T[:, m2, :], v2_sb, q2T, J // P, attn2T)

        wo2_sb = wres.tile([P, KT, INNER], f8, tag="wbig")
        nc.sync.dma_start(wo2_sb[:], wo2_d[:])
        h3T = big.tile([P, KT, N_OWN], bf16, tag="t2m")
        for tt in range(TT_OWN):
            o_proj_tile(attn2T, wo2_sb, 1, x_own[tt], tt, True)
            layernorm_tile(x_own[tt], tt, 2, dst_sb=h3T)

        # ---------------- Phase 9: GEGLU FF (bf16) ----------------
        g_sb = big.tile([P, 8, N_OWN], bf16, tag="g_sb", bufs=1)
        for grp in range(4):
            wf2g = wres.tile([P, 8, D], bf16, tag="wbig")
            nc.sync.dma_start(wf2g[:], wf2_d[:, grp * 8:(grp + 1) * 8, :])
            for j in range(8):
                f = grp * 8 + j
                wa = wsm.tile([P, KT, P], bf16, tag="wstream")
                nc.sync.dma_start(wa[:], wf1_d[:, :, f * P:(f + 1) * P])
                wg = wsm.tile([P, KT, P], bf16, tag="wstream")
                nc.sync.dma_start(wg[:], wf1_d[:, :, FF + f * P:FF + (f + 1) * P])
                gt_sb = evict.tile([P, N_OWN], bf16, tag="gt_sb")
                for qc in range(2):
                    sl = slice(qc * 512, (qc + 1) * 512)
                    ps2 = ps_sc.tile([P, 1024], f32, tag="pssc")
                    for kt in range(KT):
                        nc.tensor.matmul(ps2[:, 0:512], wg[:, kt, :], h3T[:, kt, sl],
                                         start=(kt == 0), stop=(kt == KT - 1))
                    nc.scalar.activation(gt_sb[:, sl], ps2[:, 0:512], AF.Gelu,
                                         bias=fb1_sb[:, 32 + f:32 + f + 1])
                    ps1 = ps_sc.tile([P, 1024], f32, tag="pssc")
                    for kt in range(KT):
                        nc.tensor.matmul(ps1[:, 0:512], wa[:, kt, :], h3T[:, kt, sl],
                                         start=(kt == 0), stop=(kt == KT - 1))
                    # g = (a + b1a) * gelu(gate)
                    nc.vector.scalar_tensor_tensor(
                        g_sb[:, j, sl], ps1[:, 0:512], fb1_sb[:, f:f + 1],
                        gt_sb[:, sl], ALU.add, ALU.mult)
            for tt in range(TT_OWN):
                for dc in range(2):
                    sl = slice(dc * 512, (dc + 1) * 512)
                    ps = ps_a.tile([P, 512], f32, tag="psa")
                    for jj in range(8):
                        nc.tensor.matmul(ps[:], g_sb[:, jj, tt * P:(tt + 1) * P],
                                         wf2g[:, jj, sl],
                                         start=(jj == 0),
                                         stop=(jj == 7 and grp != 3))
                    if grp < 3:
                        nc.vector.tensor_tensor(
                            x_own[tt][:, sl], x_own[tt][:, sl], ps[:], ALU.add)
                    else:
                        nc.tensor.matmul(
                            ps[:], ones1[:],
                            brow_sb[:, 2 * D + dc * 512:2 * D + (dc + 1) * 512],
                            start=False, stop=True)
                        yt = evict.tile([P, 512], f32, tag="yt", bufs=1)
                        nc.vector.tensor_tensor(yt[:], x_own[tt][:, sl], ps[:],
                                                ALU.add)
                        nc.sync.dma_start(y_d[tt, :, sl], yt[:])

    nc.compile()
    return nc


def _rearr_w(w, kt):
    return np.ascontiguousarray(
        w.reshape(kt, P, -1).transpose(1, 0, 2)).astype(BF16)


def _rearr_w8(w, kt):
    return np.ascontiguousarray(
        w.reshape(kt, P, -1).transpose(1, 0, 2) * WS).astype(F8E4)


def _shard_inputs(inputs):
    f = {k: np.asarray(v, dtype=np.float32) for k, v in inputs.items()}
    shared = {
        "nw": _rearr_w(np.concatenate([f["n1_w"], f["n2_w"], f["n3_w"]], axis=1), KT),
        "nbc": np.ascontiguousarray(
            np.concatenate([f["n1_b"], f["n2_b"], f["n3_b"]])
            .reshape(3, 16, P).transpose(2, 0, 1).reshape(P, 48)),
        "wq1": _rearr_w8(f["q1"], KT), "wk1": _rearr_w8(f["k1"], KT),
        "wv1": _rearr_w8(f["v1"], KT), "wo1": _rearr_w(f["o1_w"], KT),
        "wq2": _rearr_w8(f["q2"], KT), "wk2": _rearr_w8(f["k2"], CKT),
        "wv2": _rearr_w8(f["v2"], CKT), "wo2": _rearr_w8(f["o2_w"], KT),
        "brow": np.ascontiguousarray(
            np.concatenate([f["o1_b"], WS * f["o2_b"], f["ff_b2"]])[None]).astype(BF16),
        "fb1c": np.ascontiguousarray(f["ff_b1"].reshape(64, P).T),
        "wf1": _rearr_w(f["ff_w1"], KT),
        "wf2": _rearr_w(f["ff_w2"], FF // P),
    }
    in_maps = []
    for core in range(8):
        b, half = core // 2, core % 2
        own = f["x"][b, half * N_OWN:(half + 1) * N_OWN]
        oth = f["x"][b, (1 - half) * N_OWN:(2 - half) * N_OWN]
        m = dict(shared)
        m["xf"] = np.ascontiguousarray(
            np.concatenate([own, oth]).reshape(TT_FULL, P, D))
        m["tT"] = np.ascontiguousarray(f["t"][b, 0].reshape(KT, P).T).astype(BF16)
        m["ctxT"] = np.ascontiguousarray(
            f["context"][b].T.reshape(CKT, P, J).transpose(1, 0, 2)).astype(F8E4)
        in_maps.append(m)
    return in_maps


def kernel(**inputs):
    from concourse.bass_utils import run_bass_kernel_spmd
    if "nc" not in _CACHE:
        _CACHE["nc"] = _build_program()
    nc = _CACHE["nc"]
    in_maps = _shard_inputs(inputs)
    res = run_bass_kernel_spmd(nc, in_maps, core_ids=list(range(8)))
    out = np.empty((B, N, D), dtype=np.float32)
    for core in range(8):
        b, half = core // 2, core % 2
        out[b, half * N_OWN:(half + 1) * N_OWN] = \
            res.results[core]["y"].reshape(N_OWN, D)
    return out
